# revision 1
# baseline (speedup 1.0000x reference)
"""Trainium2 Bass kernel for nn_DecoderBlock (self-attn + cross-attn + FFN, post-LN).

Sharding: data-parallel over batch (2 groups of 4 cores), tensor-parallel over
heads / FFN hidden dim within each group. Three AllReduces per group, chunked
into token-halves and software-pipelined against compute (attention/out-proj/
FFN are emitted per token-half so each AllReduce overlaps the other half).

All on-device activations are kept feature-major ([features on partitions,
tokens on free axis]) so every linear layer consumes natural-layout weights.
The host pre-transposes tgt/memory and post-transposes the output.
"""

import os
import sys

sys.path.insert(0, "/opt/trn_rl_repo")

from contextlib import ExitStack

import numpy as np

import concourse.bacc as bacc
import concourse.tile as tile
from concourse import mybir
from concourse.bass_utils import run_bass_kernel_spmd

F32R = mybir.dt.float32r
F32 = mybir.dt.float32
AF = mybir.ActivationFunctionType
ALU = mybir.AluOpType

B = 2
D = 1024
H = 16
HD = 64
FF = 4 * D
NCORES = 8
TP = 4
HL = H // TP          # 4 local heads
DC = HL * HD          # 256 local q/k/v features
DCA = HL * (HD + 1)   # 260: V augmented with a ones column per head
FFC = FF // TP        # 1024 local ffn features
GROUPS = [[0, 1, 2, 3], [4, 5, 6, 7]]
DT = D // 128         # 8 feature partition-tiles
NF = FFC // 128       # 8 ffn partition-tiles

_nc_cache = {}


def _build(S, M, causal):
    nc = bacc.Bacc(None, target_bir_lowering=False, num_devices=NCORES)

    SH = S // 2           # tokens per pipeline half
    CW = min(512, SH)     # chunk width
    NCH = SH // CW        # chunks per half
    SC_ALL = S // CW      # chunks total

    # ---- DRAM parameters ----
    dp = nc.declare_dram_parameter
    xT = dp("xT", [D, S], F32R, isOutput=False)
    memT = dp("memT", [D, M], F32R, isOutput=False)
    wq = dp("wq", [D, DC], F32R, isOutput=False)
    wk = dp("wk", [D, DC], F32R, isOutput=False)
    wv = dp("wv", [D, DCA], F32R, isOutput=False)
    bq = dp("bq", [DC, 1], F32, isOutput=False)
    bk = dp("bk", [DC, 1], F32, isOutput=False)
    bva = dp("bva", [1, DCA], F32, isOutput=False)
    wo = dp("wo", [DC, D], F32R, isOutput=False)
    bo = dp("bo", [D, 1], F32, isOutput=False)
    wqx = dp("wqx", [D, DC], F32R, isOutput=False)
    wkx = dp("wkx", [D, DC], F32R, isOutput=False)
    wvx = dp("wvx", [D, DCA], F32R, isOutput=False)
    bqx = dp("bqx", [DC, 1], F32, isOutput=False)
    bkx = dp("bkx", [DC, 1], F32, isOutput=False)
    bvxa = dp("bvxa", [1, DCA], F32, isOutput=False)
    wox = dp("wox", [DC, D], F32R, isOutput=False)
    box = dp("box", [D, 1], F32, isOutput=False)
    w1 = dp("w1", [D, FFC], F32R, isOutput=False)
    b1 = dp("b1", [FFC, 1], F32, isOutput=False)
    w2 = dp("w2", [FFC, D], F32R, isOutput=False)
    b2 = dp("b2", [D, 1], F32, isOutput=False)
    lng = dp("lng", [3 * D, 1], F32, isOutput=False)
    lnb = dp("lnb", [3 * D, 1], F32, isOutput=False)
    ones = dp("ones", [128, 128], F32R, isOutput=False)
    outT = dp("outT", [D, S], F32R, isOutput=True)

    with tile.TileContext(nc) as tc, ExitStack() as st:
        ep = st.enter_context
        constp = ep(tc.tile_pool(name="const", bufs=1))
        actp = ep(tc.tile_pool(name="act", bufs=8))
        dramp = ep(tc.tile_pool(name="dram", bufs=1, space="DRAM"))

        DMA_ENG = [nc.sync, nc.gpsimd, nc.scalar]

        def dma_spread(out_ap, in_ap, nsplit=4):
            """Split a [P, N] DMA across engine queues by partition range."""
            p = out_ap.shape[0]
            step = p // nsplit
            for i in range(nsplit):
                sl = slice(i * step, (i + 1) * step if i < nsplit - 1 else p)
                DMA_ENG[i % len(DMA_ENG)].dma_start(out=out_ap[sl], in_=in_ap[sl])

        # ---- constants ----
        ones_t = constp.tile([128, 128], F32R, name="ones_t")
        nc.sync.dma_start(out=ones_t[:], in_=ones[:, :])
        eps_t = constp.tile([128, 1], F32, name="eps_t")
        nc.vector.memset(eps_t[:], 1e-5)

        def bias_tiles(src, n, prefix):
            ts = []
            for i in range(n):
                t = constp.tile([128, 1], F32, name=f"{prefix}{i}")
                nc.sync.dma_start(out=t[:], in_=src[i * 128:(i + 1) * 128, :])
                ts.append(t)
            return ts

        bq_t = bias_tiles(bq, 2, "bq")
        bk_t = bias_tiles(bk, 2, "bk")
        bqx_t = bias_tiles(bqx, 2, "bqx")
        bkx_t = bias_tiles(bkx, 2, "bkx")
        bo_t = bias_tiles(bo, DT, "bo")
        box_t = bias_tiles(box, DT, "box")
        b1_t = bias_tiles(b1, NF, "b1")
        b2_t = bias_tiles(b2, DT, "b2")
        bva_t = constp.tile([128, DCA], F32, name="bva_t")
        nc.gpsimd.dma_start(out=bva_t[:], in_=bva[:, :].to_broadcast([128, DCA]))
        bvxa_t = constp.tile([128, DCA], F32, name="bvxa_t")
        nc.gpsimd.dma_start(out=bvxa_t[:], in_=bvxa[:, :].to_broadcast([128, DCA]))
        lng_t = bias_tiles(lng, 3 * DT, "lng")
        lnb_t = bias_tiles(lnb, 3 * DT, "lnb")

        # ---- DRAM bounce buffers (per stage, per token-half) ----
        ccbuf = [[(dramp.tile([D, SH], F32, name=f"zin{s_}{h_}"),
                   dramp.tile([D, SH], F32, name=f"zout{s_}{h_}"))
                  for h_ in range(2)] for s_ in range(3)]

        # ================= helpers =================
        def load_w(pool, src, cols, tag, nbufs=8):
            ts = []
            for d in range(DT):
                t = pool.tile([128, cols], F32R, name=tag, tag=tag, bufs=nbufs)
                DMA_ENG[d % len(DMA_ENG)].dma_start(
                    out=t[:], in_=src[d * 128:(d + 1) * 128, :])
                ts.append(t)
            return ts

        def project_qk_half(qt_pair, w_tiles, b_tiles, src_tiles, psum, th, tag):
            """Fill token-half `th` of packed q/k tiles (2x [128, S])."""
            for t in range(2):
                for c in range(NCH):
                    sc = th * NCH + c
                    sl = slice(sc * CW, (sc + 1) * CW)
                    ps = psum.tile([128, CW], F32, name=f"{tag}ps", tag="pshared",
                                   bufs=2)
                    for d in range(DT):
                        nc.tensor.matmul(
                            out=ps[:],
                            lhsT=w_tiles[d][:, t * 128:(t + 1) * 128],
                            rhs=src_tiles[d][:, sl],
                            start=(d == 0), stop=(d == DT - 1),
                        )
                    nc.scalar.activation(out=qt_pair[t][:, sl], in_=ps[:],
                                         func=AF.Identity, bias=b_tiles[t][:],
                                         scale=1.0)

        def attention_half(apool, ppool, dpool, q_tiles, k_tiles, v_tiles,
                           n_keys, use_mask, psc, ppv, th, tag):
            """Emit attention for token-half th; returns 2 packed [128, SH]
            tiles (2 heads each)."""
            a_packed = [apool.tile([128, SH], F32R, name=f"{tag}{t}", tag="attn",
                                   bufs=4) for t in range(2)]
            kt_total = n_keys // 128
            kpc = CW // 128  # key tiles per chunk width
            for qc in range(th * NCH, (th + 1) * NCH):
                for h in range(HL):
                    par, ti = h % 2, h // 2
                    kts = range(min(kt_total, kpc * (qc + 1)) if use_mask
                                else kt_total)
                    n_kt = len(kts)
                    pv_ps = ppv.tile([65, CW], F32, name=f"{tag}pv", tag="pvps",
                                     bufs=2)
                    for i, kt in enumerate(kts):
                        s_ps = psc.tile([128, CW], F32, name=f"{tag}s",
                                        tag="scps", bufs=2)
                        nc.tensor.matmul(
                            out=s_ps[:],
                            lhsT=k_tiles[ti][par * 64:(par + 1) * 64,
                                             kt * 128:(kt + 1) * 128],
                            rhs=q_tiles[ti][par * 64:(par + 1) * 64,
                                            qc * CW:(qc + 1) * CW],
                            start=True, stop=True,
                        )
                        p_t = ppool.tile([128, CW], F32R, name=f"{tag}p",
                                         tag="p", bufs=4)
                        nc.scalar.activation(out=p_t[:], in_=s_ps[:], func=AF.Exp)
                        if use_mask and kt >= kpc * qc:
                            p_m = ppool.tile([128, CW], F32R, name=f"{tag}pm",
                                             tag="p", bufs=4)
                            nc.gpsimd.affine_select(
                                out=p_m[:], in_=p_t[:], pattern=[[1, CW]],
                                compare_op=ALU.is_ge, fill=0.0,
                                base=qc * CW - kt * 128, channel_multiplier=-1)
                            p_use = p_m
                        else:
                            p_use = p_t
                        nc.tensor.matmul(
                            out=pv_ps[:],
                            lhsT=v_tiles[kt][:, h * 65:(h + 1) * 65],
                            rhs=p_use[:],
                            start=(i == 0), stop=(i == n_kt - 1),
                        )
                    # softmax denominator: row 64 -> partition 0 -> broadcast
                    dsc = dpool.tile([65, CW], F32, name=f"{tag}dsc", tag="dsc",
                                     bufs=1)
                    nc.scalar.activation(out=dsc[64:65, :], in_=pv_ps[64:65, :],
                                         func=AF.Identity)
                    dn0 = dpool.tile([1, CW], F32, name=f"{tag}dn0", tag="dn0",
                                     bufs=1)
                    nc.sync.dma_start(out=dn0[0:1, :], in_=dsc[64:65, :])
                    db = dpool.tile([64, CW], F32, name=f"{tag}db", tag="db",
                                    bufs=1)
                    nc.gpsimd.partition_broadcast(db[:], dn0[0:1, :])
                    nc.vector.reciprocal(out=db[:], in_=db[:])
                    lc = qc - th * NCH
                    sl = slice(lc * CW, (lc + 1) * CW)
                    if par == 0:
                        nc.vector.tensor_mul(out=a_packed[ti][0:64, sl],
                                             in0=pv_ps[0:64, :], in1=db[:])
                    else:
                        sh = dpool.tile([64, CW], F32R, name=f"{tag}sh",
                                        tag="sh", bufs=1)
                        nc.vector.tensor_mul(out=sh[:], in0=pv_ps[0:64, :],
                                             in1=db[:])
                        nc.sync.dma_start(out=a_packed[ti][64:128, sl], in_=sh[:])
            return a_packed

        def out_project_half(wo_t, zpool, a_packed, bo_tiles, stage, psum, th,
                             tag):
            """y = wo.T @ attn for token-half th; spill + AllReduce."""
            zin, zout = ccbuf[stage][th]
            for d in range(DT):
                zp = zpool.tile([128, SH], F32, name=f"{tag}zp", tag="zp",
                                bufs=2)
                for c in range(NCH):
                    sc = th * NCH + c
                    ps = psum.tile([128, CW], F32, name=f"{tag}ps", tag="ops",
                                   bufs=2)
                    for ct in range(2):
                        nc.tensor.matmul(
                            out=ps[:], lhsT=wo_t[(ct, d)][:],
                            rhs=a_packed[ct][:, c * CW:(c + 1) * CW],
                            start=(ct == 0), stop=(ct == 1),
                        )
                    nc.scalar.activation(
                        out=zp[:, c * CW:(c + 1) * CW], in_=ps[:],
                        func=AF.Identity, bias=bo_tiles[d][:], scale=1.0)
                dma_spread(zin[d * 128:(d + 1) * 128, :], zp[:])
            nc.gpsimd.collective_compute(
                "AllReduce", ALU.add, replica_groups=GROUPS,
                ins=[zin.opt()], outs=[zout.opt()])

        def load_wo(wpool, wo_src, tag):
            wo_t = {}
            for ct in range(2):
                for d in range(DT):
                    t = wpool.tile([128, 128], F32R, name=f"{tag}w", tag="wo",
                                   bufs=16)
                    DMA_ENG[d % len(DMA_ENG)].dma_start(
                        out=t[:], in_=wo_src[ct * 128:(ct + 1) * 128,
                                             d * 128:(d + 1) * 128])
                    wo_t[(ct, d)] = t
            return wo_t

        def reload_half(zrpool, res_tiles, stage, th, tag):
            """z[:, half] = allreduced + residual, in place over res tiles."""
            hs = slice(th * SH, (th + 1) * SH)
            for d in range(DT):
                zr = zrpool.tile([128, SH], F32, name=f"{tag}zr", tag="zr",
                                 bufs=2)
                dma_spread(zr[:], ccbuf[stage][th][1][d * 128:(d + 1) * 128, :])
                nc.vector.tensor_add(out=res_tiles[d][:, hs], in0=zr[:],
                                     in1=res_tiles[d][:, hs].bitcast(F32))

        def ln_half(sqpool, tmppool, z_tiles, ln_idx, psum, th, tag):
            for c in range(NCH):
                ch = th * NCH + c
                sl = slice(ch * CW, (ch + 1) * CW)
                mps = psum.tile([128, CW], F32, name=f"{tag}m", tag="pshared",
                                bufs=2)
                for d in range(DT):
                    nc.tensor.matmul(out=mps[:], lhsT=ones_t[:],
                                     rhs=z_tiles[d][:, sl],
                                     start=(d == 0), stop=(d == DT - 1))
                mu = tmppool.tile([128, CW], F32, name=f"{tag}mu", tag="mu",
                                  bufs=2)
                nc.scalar.copy(out=mu[:], in_=mps[:])
                qps = psum.tile([128, CW], F32, name=f"{tag}q", tag="pshared",
                                bufs=2)
                for d in range(DT):
                    sq = sqpool.tile([128, CW], F32R, name=f"{tag}sq", tag="sq",
                                     bufs=2)
                    nc.scalar.activation(out=sq[:], in_=z_tiles[d][:, sl],
                                         func=AF.Square)
                    nc.tensor.matmul(out=qps[:], lhsT=ones_t[:], rhs=sq[:],
                                     start=(d == 0), stop=(d == DT - 1))
                var = tmppool.tile([128, CW], F32, name=f"{tag}v", tag="t",
                                   bufs=2)
                nc.vector.tensor_mul(out=var[:], in0=mu[:], in1=mu[:])
                nc.vector.tensor_tensor(out=var[:], in0=qps[:], in1=var[:],
                                        op=ALU.subtract)
                std = tmppool.tile([128, CW], F32, name=f"{tag}st", tag="t",
                                   bufs=2)
                nc.scalar.activation(out=std[:], in_=var[:], func=AF.Sqrt,
                                     bias=eps_t[:], scale=1.0)
                rstd = tmppool.tile([128, CW], F32, name=f"{tag}r", tag="rstd",
                                    bufs=2)
                nc.vector.reciprocal(out=rstd[:], in_=std[:])
                for d in range(DT):
                    xm = tmppool.tile([128, CW], F32, name=f"{tag}x", tag="t",
                                      bufs=2)
                    nc.vector.tensor_tensor(out=xm[:],
                                            in0=z_tiles[d][:, sl].bitcast(F32),
                                            in1=mu[:], op=ALU.subtract)
                    nc.vector.tensor_mul(out=xm[:], in0=xm[:], in1=rstd[:])
                    nc.vector.tensor_scalar(
                        out=z_tiles[d][:, sl], in0=xm[:],
                        scalar1=lng_t[ln_idx * DT + d][:],
                        scalar2=lnb_t[ln_idx * DT + d][:],
                        op0=ALU.mult, op1=ALU.add)

        # ================= pipeline =================
        # ---- P1: load x, self QKV ----
        x_fm = []
        for d in range(DT):
            t = actp.tile([128, S], F32R, name="xfm", tag="act")
            dma_spread(t[:], xT[d * 128:(d + 1) * 128, :])
            x_fm.append(t)

        attn_stack = ExitStack()
        ap2 = attn_stack.enter_context
        qkp = ap2(tc.tile_pool(name="qk", bufs=4))
        vp = ap2(tc.tile_pool(name="vp", bufs=16))
        attnp = ap2(tc.tile_pool(name="attn", bufs=2))
        w_stack = ExitStack()
        wqkp = w_stack.enter_context(tc.tile_pool(name="wqk", bufs=8))
        wvp = w_stack.enter_context(tc.tile_pool(name="wvp", bufs=8))

        wq_t = load_w(wqkp, wq, DC, "wqt")
        wk_t = load_w(wqkp, wk, DC, "wkt")
        wv_t = load_w(wvp, wv, DCA, "wvt")

        q_s = [qkp.tile([128, S], F32R, name="qs", tag="qk", bufs=4)
               for _ in range(2)]
        k_s = [qkp.tile([128, S], F32R, name="ks", tag="qk", bufs=4)
               for _ in range(2)]
        with tc.tile_pool(name="ps1", bufs=2, space="PSUM") as ps1, \
             tc.tile_pool(name="ps1v", bufs=2, space="PSUM") as ps1v:
            for th in range(2):
                project_qk_half(q_s, wq_t, bq_t, x_fm, ps1, th, "qs")
                project_qk_half(k_s, wk_t, bk_t, x_fm, ps1, th, "ks")
            v_s = []
            for s_t in range(S // 128):
                vt = vp.tile([128, DCA], F32R, name="vs", tag="v", bufs=16)
                ps = ps1v.tile([128, DCA], F32, name="vps", tag="vps", bufs=2)
                for d in range(DT):
                    nc.tensor.matmul(
                        out=ps[:],
                        lhsT=x_fm[d][:, s_t * 128:(s_t + 1) * 128],
                        rhs=wv_t[d][:],
                        start=(d == 0), stop=(d == DT - 1))
                nc.vector.tensor_add(out=vt[:], in0=ps[:], in1=bva_t[:])
                v_s.append(vt)

        # ---- P2/P3: self attention + out-proj + CC, per token-half ----
        with tc.tile_pool(name="pp1", bufs=5) as pp1, \
             tc.tile_pool(name="dn1", bufs=2) as dn1, \
             tc.tile_pool(name="wo1", bufs=16) as wo1p, \
             tc.tile_pool(name="zp1", bufs=3) as zp1, \
             tc.tile_pool(name="ps2s", bufs=3, space="PSUM") as ps2s, \
             tc.tile_pool(name="ps2v", bufs=2, space="PSUM") as ps2v, \
             tc.tile_pool(name="ps3", bufs=2, space="PSUM") as ps3:
            wo_t = load_wo(wo1p, wo, "o1")
            for th in range(2):
                a_h = attention_half(attnp, pp1, dn1, q_s, k_s, v_s, S, causal,
                                     ps2s, ps2v, th, "sa")
                out_project_half(wo_t, zp1, a_h, bo_t, 0, ps3, th, "o1")

        # ---- P4: cross K/V (fills CC1 window) + x reload ----
        wkx_t = load_w(wqkp, wkx, DC, "wkt")
        wvx_t = load_w(wvp, wvx, DCA, "wvt")
        k_x = [qkp.tile([128, M], F32R, name="kx", tag="qk", bufs=4)
               for _ in range(2)]
        with tc.tile_pool(name="memp", bufs=3) as memp, \
             tc.tile_pool(name="ps4k", bufs=8, space="PSUM") as ps4k:
            kx_ps = {}
            for d in range(DT):
                mt = memp.tile([128, M], F32R, name="memt", tag="mem", bufs=3)
                dma_spread(mt[:], memT[d * 128:(d + 1) * 128, :])
                for t in range(2):
                    for sc in range(M // 512):
                        if d == 0:
                            kx_ps[(t, sc)] = ps4k.tile(
                                [128, 512], F32, name="kxps", tag="kxps", bufs=8)
                        nc.tensor.matmul(
                            out=kx_ps[(t, sc)][:],
                            lhsT=wkx_t[d][:, t * 128:(t + 1) * 128],
                            rhs=mt[:, sc * 512:(sc + 1) * 512],
                            start=(d == 0), stop=(d == DT - 1))
            for t in range(2):
                for sc in range(M // 512):
                    nc.scalar.activation(
                        out=k_x[t][:, sc * 512:(sc + 1) * 512],
                        in_=kx_ps[(t, sc)][:], func=AF.Identity,
                        bias=bkx_t[t][:], scale=1.0)

        MT = M // 128
        v_x = [None] * MT
        with tc.tile_pool(name="memp2", bufs=3) as memp2, \
             tc.tile_pool(name="ps4v", bufs=8, space="PSUM") as ps4v:
            for g in range((MT + 7) // 8):
                sts = range(g * 8, min((g + 1) * 8, MT))
                vx_ps = {}
                for d in range(DT):
                    mt = memp2.tile([128, M], F32R, name="memt2", tag="mem",
                                    bufs=3)
                    dma_spread(mt[:], memT[d * 128:(d + 1) * 128, :])
                    for s_t in sts:
                        if d == 0:
                            vx_ps[s_t] = ps4v.tile([128, DCA], F32, name="vxps",
                                                   tag="vxps", bufs=8)
                        nc.tensor.matmul(
                            out=vx_ps[s_t][:],
                            lhsT=mt[:, s_t * 128:(s_t + 1) * 128],
                            rhs=wvx_t[d][:],
                            start=(d == 0), stop=(d == DT - 1))
                for s_t in sts:
                    vt = vp.tile([128, DCA], F32R, name="vx", tag="v", bufs=16)
                    nc.vector.tensor_add(out=vt[:], in0=vx_ps[s_t][:],
                                         in1=bvxa_t[:])
                    v_x[s_t] = vt

        w_stack.close()

        # ---- P5..P8: LN1, cross attention, cross out-proj + CC2 ----
        q_x = [qkp.tile([128, S], F32R, name="qx", tag="qk", bufs=4)
               for _ in range(2)]
        agg1 = x_fm
        with tc.tile_pool(name="zrA", bufs=4) as zrA, \
             tc.tile_pool(name="sqA", bufs=2) as sqA, \
             tc.tile_pool(name="tmA", bufs=3) as tmA, \
             tc.tile_pool(name="pp2", bufs=5) as pp2, \
             tc.tile_pool(name="dn2", bufs=2) as dn2, \
             tc.tile_pool(name="wo2", bufs=16) as wo2p, \
             tc.tile_pool(name="zp2", bufs=3) as zp2, \
             tc.tile_pool(name="ps56", bufs=2, space="PSUM") as ps56, \
             tc.tile_pool(name="ps6s", bufs=2, space="PSUM") as ps6s, \
             tc.tile_pool(name="ps6v", bufs=2, space="PSUM") as ps6v, \
             tc.tile_pool(name="wqxp", bufs=8) as wqxp, \
             tc.tile_pool(name="ps7", bufs=2, space="PSUM") as ps7:
            wox_t = load_wo(wo2p, wox, "o2")
            wqx_t = load_w(wqxp, wqx, DC, "wqxt")
            for th in range(2):
                reload_half(zrA, agg1, 0, th, "l1")
                ln_half(sqA, tmA, agg1, 0, ps56, th, "l1")
                project_qk_half(q_x, wqx_t, bqx_t, agg1, ps56, th, "qx")
            for th in range(2):
                a_h = attention_half(attnp, pp2, dn2, q_x, k_x, v_x, M, False,
                                     ps6s, ps6v, th, "cx")
                out_project_half(wox_t, zp2, a_h, box_t, 1, ps7, th, "o2")

        attn_stack.close()

        # ---- P9..P11: LN2, FFN (per half) + CC3 ----
        agg2 = agg1
        with tc.tile_pool(name="zrB", bufs=4) as zrB, \
             tc.tile_pool(name="sqB", bufs=2) as sqB, \
             tc.tile_pool(name="tmB", bufs=3) as tmB, \
             tc.tile_pool(name="hfm", bufs=8) as hfmp, \
             tc.tile_pool(name="wf", bufs=16) as wfp, \
             tc.tile_pool(name="zp3", bufs=3) as zp3, \
             tc.tile_pool(name="ps8", bufs=2, space="PSUM") as ps8, \
             tc.tile_pool(name="ps9a", bufs=2, space="PSUM") as ps9a, \
             tc.tile_pool(name="ps9b", bufs=2, space="PSUM") as ps9b:
            h_fm = [hfmp.tile([128, S], F32R, name=f"hfm{f}", tag="hfm", bufs=8)
                    for f in range(NF)]
            for th in range(2):
                reload_half(zrB, agg2, 1, th, "l2")
                ln_half(sqB, tmB, agg2, 1, ps8, th, "l2")
                # FFN layer 1 for this half
                for f in range(NF):
                    w1_t = []
                    for d in range(DT):
                        t = wfp.tile([128, 128], F32R, name="w1t", tag="wf",
                                     bufs=16)
                        DMA_ENG[d % len(DMA_ENG)].dma_start(
                            out=t[:],
                            in_=w1[d * 128:(d + 1) * 128, f * 128:(f + 1) * 128])
                        w1_t.append(t)
                    for c in range(NCH):
                        sc = th * NCH + c
                        sl = slice(sc * CW, (sc + 1) * CW)
                        ps = ps9a.tile([128, CW], F32, name="f1ps", tag="f1ps",
                                       bufs=2)
                        for d in range(DT):
                            nc.tensor.matmul(
                                out=ps[:], lhsT=w1_t[d][:],
                                rhs=agg2[d][:, sl],
                                start=(d == 0), stop=(d == DT - 1))
                        nc.scalar.activation(out=h_fm[f][:, sl], in_=ps[:],
                                             func=AF.Relu, bias=b1_t[f][:],
                                             scale=1.0)
                # FFN layer 2 for this half + spill + CC
                zin, zout = ccbuf[2][th]
                for d in range(DT):
                    w2_t = []
                    for f in range(NF):
                        t = wfp.tile([128, 128], F32R, name="w2t", tag="wf2",
                                     bufs=16)
                        DMA_ENG[f % len(DMA_ENG)].dma_start(
                            out=t[:],
                            in_=w2[f * 128:(f + 1) * 128, d * 128:(d + 1) * 128])
                        w2_t.append(t)
                    zp = zp3.tile([128, SH], F32, name="f2zp", tag="zp", bufs=2)
                    for c in range(NCH):
                        sc = th * NCH + c
                        ps = ps9b.tile([128, CW], F32, name="f2ps", tag="f2ps",
                                       bufs=2)
                        for f in range(NF):
                            nc.tensor.matmul(
                                out=ps[:], lhsT=w2_t[f][:],
                                rhs=h_fm[f][:, sc * CW:(sc + 1) * CW],
                                start=(f == 0), stop=(f == NF - 1))
                        nc.scalar.activation(
                            out=zp[:, c * CW:(c + 1) * CW], in_=ps[:],
                            func=AF.Identity, bias=b2_t[d][:], scale=1.0)
                    dma_spread(zin[d * 128:(d + 1) * 128, :], zp[:])
                nc.gpsimd.collective_compute(
                    "AllReduce", ALU.add, replica_groups=GROUPS,
                    ins=[zin.opt()], outs=[zout.opt()])

        # ---- P12: LN3 + output ----
        agg3 = agg2
        with tc.tile_pool(name="zrC", bufs=4) as zrC, \
             tc.tile_pool(name="sqC", bufs=2) as sqC, \
             tc.tile_pool(name="tmC", bufs=3) as tmC, \
             tc.tile_pool(name="ps10", bufs=2, space="PSUM") as ps10:
            for th in range(2):
                reload_half(zrC, agg3, 2, th, "l3")
                ln_half(sqC, tmC, agg3, 2, ps10, th, "l3")
                hs = slice(th * SH, (th + 1) * SH)
                for d in range(DT):
                    dma_spread(outT[d * 128:(d + 1) * 128, hs],
                               agg3[d][:, hs])

    nc.finalize()
    return nc


def _get_nc(S, M, causal):
    key = (S, M, causal)
    if key not in _nc_cache:
        _nc_cache[key] = _build(S, M, causal)
    return _nc_cache[key]


def _prep_inputs(c, S, M, tgt, memory, Wqkv, bqkv, Wo_sa, bo_sa, Wq, bq, Wk, bk,
                 Wv, bv, Wo_cx, bo_cx, W1, b1, W2, b2, g_mha, bn_mha, g_crx,
                 bn_crx, g_ffn, bn_ffn):
    r, b = c % TP, c // TP
    hsl = slice(r * DC, (r + 1) * DC)
    fsl = slice(r * FFC, (r + 1) * FFC)
    f32 = np.float32

    def aug_v(wv_c, bv_c):
        wva = np.zeros((D, DCA), f32)
        bva = np.zeros((1, DCA), f32)
        for h in range(HL):
            wva[:, h * 65:h * 65 + 64] = wv_c[:, h * 64:(h + 1) * 64]
            bva[0, h * 65:h * 65 + 64] = bv_c[h * 64:(h + 1) * 64]
            bva[0, h * 65 + 64] = 1.0
        return wva, bva

    scale = np.float32(1.0 / np.sqrt(HD))
    # Wqkv columns are per-head interleaved: head g = cols g*192 + [q64|k64|v64]
    wqkv_h = Wqkv.reshape(D, H, 3 * HD)
    bqkv_h = bqkv.reshape(H, 3 * HD)
    gh = slice(r * HL, (r + 1) * HL)  # this rank's global heads
    wq_sa = wqkv_h[:, gh, 0:HD].reshape(D, DC) * scale
    wk_sa = wqkv_h[:, gh, HD:2 * HD].reshape(D, DC)
    wv_sa = wqkv_h[:, gh, 2 * HD:3 * HD].reshape(D, DC)
    bq_sa = bqkv_h[gh, 0:HD].reshape(DC) * scale
    bk_sa = bqkv_h[gh, HD:2 * HD].reshape(DC)
    bv_sa = bqkv_h[gh, 2 * HD:3 * HD].reshape(DC)
    wva_sa, bva_sa = aug_v(wv_sa, bv_sa)
    wvx_c, bvx_c = aug_v(Wv[:, hsl], bv[hsl])
    rank0 = np.float32(1.0 if r == 0 else 0.0)
    return {
        "xT": np.ascontiguousarray(tgt[b].T, f32),
        "memT": np.ascontiguousarray(memory[b].T, f32),
        "wq": np.ascontiguousarray(wq_sa, f32),
        "wk": np.ascontiguousarray(wk_sa, f32),
        "wv": wva_sa,
        "bq": np.ascontiguousarray(bq_sa.reshape(DC, 1), f32),
        "bk": np.ascontiguousarray(bk_sa.reshape(DC, 1), f32),
        "bva": bva_sa,
        "wo": np.ascontiguousarray(Wo_sa[hsl, :], f32),
        "bo": np.ascontiguousarray((bo_sa * rank0).reshape(D, 1), f32),
        "wqx": np.ascontiguousarray(Wq[:, hsl] * scale, f32),
        "wkx": np.ascontiguousarray(Wk[:, hsl], f32),
        "wvx": wvx_c,
        "bqx": np.ascontiguousarray((bq[hsl] * scale).reshape(DC, 1), f32),
        "bkx": np.ascontiguousarray(bk[hsl].reshape(DC, 1), f32),
        "bvxa": bvx_c,
        "wox": np.ascontiguousarray(Wo_cx[hsl, :], f32),
        "box": np.ascontiguousarray((bo_cx * rank0).reshape(D, 1), f32),
        "w1": np.ascontiguousarray(W1[:, fsl], f32),
        "b1": np.ascontiguousarray(b1[fsl].reshape(FFC, 1), f32),
        "w2": np.ascontiguousarray(W2[fsl, :], f32),
        "b2": np.ascontiguousarray((b2 * rank0).reshape(D, 1), f32),
        "lng": np.ascontiguousarray(
            np.concatenate([g_mha, g_crx, g_ffn]).reshape(3 * D, 1), f32),
        "lnb": np.ascontiguousarray(
            np.concatenate([bn_mha, bn_crx, bn_ffn]).reshape(3 * D, 1), f32),
        "ones": np.full((128, 128), 1.0 / D, f32),
    }


def kernel(**inputs):
    tgt = np.asarray(inputs["tgt"], np.float32)
    memory = np.asarray(inputs["memory"], np.float32)
    mask = np.asarray(inputs["tgt_mask"])
    S, M = tgt.shape[1], memory.shape[1]

    if mask.any():
        expect = np.triu(np.ones((S, S), bool), 1)
        if not np.array_equal(mask, expect):
            raise NotImplementedError("only causal or empty tgt_mask supported")
        causal = True
    else:
        causal = False

    nc = _get_nc(S, M, causal)
    args = {k: np.asarray(v, np.float32) for k, v in inputs.items()
            if k not in ("tgt", "memory", "tgt_mask")}
    in_maps = [_prep_inputs(c, S, M, tgt, memory, **args) for c in range(NCORES)]

    trace = bool(int(os.environ.get("BASS_KERNEL_TRACE", "0")))
    res = run_bass_kernel_spmd(nc, in_maps, list(range(NCORES)), trace=trace)
    if trace:
        kernel.last_exec_time_ns = res.exec_time_ns
    out = np.stack([
        np.ascontiguousarray(res.results[0]["outT"].T),
        np.ascontiguousarray(res.results[TP]["outT"].T),
    ])
    return out.astype(np.float32)



# revision 17
# speedup vs baseline: 1.2428x; 1.2428x over previous
"""Trainium2 Bass kernel for nn_DecoderBlock (self-attn + cross-attn + FFN, post-LN).

Sharding: data-parallel over batch (2 groups of 4 cores), tensor-parallel over
heads / FFN hidden dim within each group. Three AllReduces per group, chunked
into token-halves and software-pipelined so each AllReduce overlaps the other
half's compute (including the next stage's work for the already-reduced half).

Precision: fp32 residual stream + LayerNorm; bf16 weights/attention/FFN-hidden
and bf16 AllReduce payloads (validated ~2e-3 rel err vs fp64 reference).

All on-device activations are feature-major ([features on partitions, tokens
on free axis]). The host pre-transposes tgt/memory and post-transposes out.
"""

import os
import sys

sys.path.insert(0, "/opt/trn_rl_repo")

from contextlib import ExitStack

import numpy as np
import ml_dtypes

import concourse.bacc as bacc
import concourse.tile as tile
from concourse import mybir
from concourse.bass_utils import run_bass_kernel_spmd

F32R = mybir.dt.float32r
F32 = mybir.dt.float32
BF16 = mybir.dt.bfloat16
AF = mybir.ActivationFunctionType
ALU = mybir.AluOpType

B = 2
D = 1024
H = 16
HD = 64
FF = 4 * D
NCORES = 8
TP = 4
HL = H // TP          # 4 local heads
DC = HL * HD          # 256 local q/k/v features
DCA = HL * (HD + 1)   # 260: V augmented with a ones column per head
FFC = FF // TP        # 1024 local ffn features
GROUPS = [[0, 1, 2, 3], [4, 5, 6, 7]]
DT = D // 128         # 8 feature partition-tiles
NF = FFC // 128       # 8 ffn partition-tiles

_nc_cache = {}


def _build(S, M, causal):
    nc = bacc.Bacc(None, target_bir_lowering=False, num_devices=NCORES)

    SH = S // 2           # tokens per pipeline half
    CW = 512              # chunk width
    NCH = SH // CW        # chunks per half
    KPC = CW // 128       # key tiles per chunk width

    # ---- DRAM parameters ----
    dp = nc.declare_dram_parameter
    xT = dp("xT", [D, S], F32R, isOutput=False)
    memT = dp("memT", [D, M], BF16, isOutput=False)
    wq = dp("wq", [D, DC], F32R, isOutput=False)
    wk = dp("wk", [D, DC], F32R, isOutput=False)
    wv = dp("wv", [D, DCA], F32R, isOutput=False)
    bq = dp("bq", [DC, 1], F32, isOutput=False)
    bk = dp("bk", [DC, 1], F32, isOutput=False)
    bva = dp("bva", [1, DCA], F32, isOutput=False)
    wo = dp("wo", [DC, D], BF16, isOutput=False)
    bo = dp("bo", [D, 1], F32, isOutput=False)
    wqx = dp("wqx", [D, DC], F32R, isOutput=False)
    wkx = dp("wkx", [D, DC], BF16, isOutput=False)
    wvx = dp("wvx", [D, DCA], BF16, isOutput=False)
    bqx = dp("bqx", [DC, 1], F32, isOutput=False)
    bkx = dp("bkx", [DC, 1], F32, isOutput=False)
    bvxa = dp("bvxa", [1, DCA], F32, isOutput=False)
    wox = dp("wox", [DC, D], BF16, isOutput=False)
    box = dp("box", [D, 1], F32, isOutput=False)
    w1 = dp("w1", [D, FFC], F32R, isOutput=False)
    b1 = dp("b1", [FFC, 1], F32, isOutput=False)
    w2 = dp("w2", [FFC, D], BF16, isOutput=False)
    b2 = dp("b2", [D, 1], F32, isOutput=False)
    lng = dp("lng", [3 * D, 1], F32, isOutput=False)
    lnb = dp("lnb", [3 * D, 1], F32, isOutput=False)
    ones = dp("ones", [128, 128], F32R, isOutput=False)
    outT = dp("outT", [D, S], F32R, isOutput=True)

    with tile.TileContext(nc) as tc, ExitStack() as st:
        ep = st.enter_context
        constp = ep(tc.tile_pool(name="const", bufs=1))
        aggp = ep(tc.tile_pool(name="agg", bufs=8))
        wf32p = ep(tc.tile_pool(name="wf32", bufs=8))
        wobigp = ep(tc.tile_pool(name="wobig", bufs=2))
        wsmp = ep(tc.tile_pool(name="wsm", bufs=16))
        w2p = ep(tc.tile_pool(name="w2p", bufs=8))
        qkp = ep(tc.tile_pool(name="qk", bufs=4))
        vp = ep(tc.tile_pool(name="vp", bufs=17))
        memp = ep(tc.tile_pool(name="memp", bufs=8))
        hp = ep(tc.tile_pool(name="hp", bufs=8))
        app = ep(tc.tile_pool(name="ap", bufs=3))
        ppool = ep(tc.tile_pool(name="pp", bufs=4))
        zpp = ep(tc.tile_pool(name="zpp", bufs=2))
        zrp = ep(tc.tile_pool(name="zrp", bufs=2))
        lnp = ep(tc.tile_pool(name="lnp", bufs=1))
        dnp = ep(tc.tile_pool(name="dnp", bufs=2))
        dramp = ep(tc.tile_pool(name="dram", bufs=1, space="DRAM"))
        ps_proj = ep(tc.tile_pool(name="psP", bufs=2, space="PSUM"))
        ps_sc = ep(tc.tile_pool(name="psS", bufs=2, space="PSUM"))
        ps_pv = ep(tc.tile_pool(name="psV", bufs=2, space="PSUM"))
        ps_out = ep(tc.tile_pool(name="psO", bufs=2, space="PSUM"))

        DMA_ENG = [nc.sync, nc.gpsimd, nc.scalar]

        def dma_spread(out_ap, in_ap, nsplit=4):
            p = out_ap.shape[0]
            step = p // nsplit
            for i in range(nsplit):
                sl = slice(i * step, (i + 1) * step if i < nsplit - 1 else p)
                DMA_ENG[i % len(DMA_ENG)].dma_start(out=out_ap[sl], in_=in_ap[sl])

        # ---- constants ----
        ones_t = constp.tile([128, 128], F32R, name="ones_t")
        nc.sync.dma_start(out=ones_t[:], in_=ones[:, :])
        eps_t = constp.tile([128, 1], F32, name="eps_t")
        nc.vector.memset(eps_t[:], 1e-5)

        def bias_tiles(src, n, prefix):
            ts = []
            for i in range(n):
                t = constp.tile([128, 1], F32, name=f"{prefix}{i}")
                nc.sync.dma_start(out=t[:], in_=src[i * 128:(i + 1) * 128, :])
                ts.append(t)
            return ts

        bq_t = bias_tiles(bq, 2, "bq")
        bk_t = bias_tiles(bk, 2, "bk")
        bqx_t = bias_tiles(bqx, 2, "bqx")
        bkx_t = bias_tiles(bkx, 2, "bkx")
        bo_t = bias_tiles(bo, DT, "bo")
        box_t = bias_tiles(box, DT, "box")
        b1_t = bias_tiles(b1, NF, "b1")
        b2_t = bias_tiles(b2, DT, "b2")
        bva_t = constp.tile([128, DCA], F32, name="bva_t")
        nc.gpsimd.dma_start(out=bva_t[:], in_=bva[:, :].to_broadcast([128, DCA]))
        bvxa_t = constp.tile([128, DCA], F32, name="bvxa_t")
        nc.gpsimd.dma_start(out=bvxa_t[:], in_=bvxa[:, :].to_broadcast([128, DCA]))
        lng_t = bias_tiles(lng, 3 * DT, "lng")
        lnb_t = bias_tiles(lnb, 3 * DT, "lnb")

        # ---- DRAM bounce buffers: unique tag per tensor (no aliasing) ----
        ccbuf = [[(dramp.tile([D, SH], BF16, name=f"zin{s_}{h_}",
                              tag=f"zin{s_}{h_}"),
                   dramp.tile([D, SH], BF16, name=f"zout{s_}{h_}",
                              tag=f"zout{s_}{h_}"))
                  for h_ in range(2)] for s_ in range(3)]

        # ================= helpers =================
        def load_wf32(src, cols, tag="w32"):
            """8 [128, cols] f32r tiles from the shared f32 weight ring."""
            ts = []
            for d in range(DT):
                t = wf32p.tile([128, cols], F32R, name=tag, tag="w32", bufs=8)
                DMA_ENG[d % len(DMA_ENG)].dma_start(
                    out=t[:], in_=src[d * 128:(d + 1) * 128, :])
                ts.append(t)
            return ts

        def load_wbf(pool, src, rows, cols, tag, bufs):
            """rows//128 [128, cols] bf16 tiles from a bf16 weight ring."""
            ts = []
            for d in range(rows // 128):
                t = pool.tile([128, cols], BF16, name=tag, tag=tag, bufs=bufs)
                DMA_ENG[d % len(DMA_ENG)].dma_start(
                    out=t[:], in_=src[d * 128:(d + 1) * 128, :])
                ts.append(t)
            return ts

        def project_qk(qt_pair, w_tiles, b_tiles, src_tiles, tag):
            """Full-S q/k projection into 2 packed [128, S] bf16 tiles."""
            for t in range(2):
                for sc in range(S // CW):
                    sl = slice(sc * CW, (sc + 1) * CW)
                    ps = ps_proj.tile([128, CW], F32, name=f"{tag}ps",
                                      tag="psP", bufs=2)
                    for d in range(DT):
                        nc.tensor.matmul(
                            out=ps[:],
                            lhsT=w_tiles[d][:, t * 128:(t + 1) * 128],
                            rhs=src_tiles[d][:, sl],
                            start=(d == 0), stop=(d == DT - 1),
                        )
                    nc.scalar.activation(out=qt_pair[t][:, sl], in_=ps[:],
                                         func=AF.Identity, bias=b_tiles[t][:],
                                         scale=1.0)

        def v_project_sa(w_tiles, src_tiles):
            """Self-attn V: token-major [128, DCA] bf16 tiles, one per 128 toks."""
            vs = []
            for s_t in range(S // 128):
                ps = ps_pv.tile([128, DCA], F32, name="vps", tag="psV", bufs=2)
                for d in range(DT):
                    nc.tensor.matmul(
                        out=ps[:],
                        lhsT=src_tiles[d][:, s_t * 128:(s_t + 1) * 128],
                        rhs=w_tiles[d][:],
                        start=(d == 0), stop=(d == DT - 1))
                vt = vp.tile([128, DCA], BF16, name="vs", tag="v", bufs=18)
                nc.vector.tensor_add(out=vt[:], in0=ps[:], in1=bva_t[:])
                vs.append(vt)
            return vs

        def attention_half(q_pair, k_pair, v_tiles, n_keys, use_mask, th, tag):
            """Attention for token-half th -> 2 packed [128, SH] bf16 tiles."""
            a_packed = [app.tile([128, SH], BF16, name=f"{tag}{t}", tag="attn",
                                 bufs=3) for t in range(2)]
            kt_total = n_keys // 128
            for qc in range(th * NCH, (th + 1) * NCH):
                for h in range(HL):
                    par, ti = h % 2, h // 2
                    kts = range(min(kt_total, KPC * (qc + 1)) if use_mask
                                else kt_total)
                    n_kt = len(kts)
                    pv_ps = ps_pv.tile([65, CW], F32, name=f"{tag}pv",
                                       tag="psV", bufs=2)
                    for i, kt in enumerate(kts):
                        s_ps = ps_sc.tile([128, CW], F32, name=f"{tag}s",
                                          tag="psS", bufs=2)
                        nc.tensor.matmul(
                            out=s_ps[:],
                            lhsT=k_pair[ti][par * 64:(par + 1) * 64,
                                            kt * 128:(kt + 1) * 128],
                            rhs=q_pair[ti][par * 64:(par + 1) * 64,
                                           qc * CW:(qc + 1) * CW],
                            start=True, stop=True,
                        )
                        p_t = ppool.tile([128, CW], BF16, name=f"{tag}p",
                                         tag="p", bufs=3)
                        nc.scalar.activation(out=p_t[:], in_=s_ps[:], func=AF.Exp)
                        if use_mask and kt >= KPC * qc:
                            p_m = ppool.tile([128, CW], BF16, name=f"{tag}pm",
                                             tag="p", bufs=3)
                            nc.gpsimd.affine_select(
                                out=p_m[:], in_=p_t[:], pattern=[[1, CW]],
                                compare_op=ALU.is_ge, fill=0.0,
                                base=qc * CW - kt * 128, channel_multiplier=-1)
                            p_use = p_m
                        else:
                            p_use = p_t
                        nc.tensor.matmul(
                            out=pv_ps[:],
                            lhsT=v_tiles[kt][:, h * 65:(h + 1) * 65],
                            rhs=p_use[:],
                            start=(i == 0), stop=(i == n_kt - 1),
                        )
                    # softmax denominator: row 64 -> partition 0 -> broadcast
                    dsc = dnp.tile([65, CW], F32, name=f"{tag}dsc", tag="dsc",
                                   bufs=1)
                    nc.scalar.activation(out=dsc[64:65, :], in_=pv_ps[64:65, :],
                                         func=AF.Identity)
                    dn0 = dnp.tile([1, CW], F32, name=f"{tag}dn0", tag="dn0",
                                   bufs=2)
                    nc.sync.dma_start(out=dn0[0:1, :], in_=dsc[64:65, :])
                    db = dnp.tile([64, CW], F32, name=f"{tag}db", tag="db",
                                  bufs=2)
                    nc.gpsimd.partition_broadcast(db[:], dn0[0:1, :])
                    nc.vector.reciprocal(out=db[:], in_=db[:])
                    lc = qc - th * NCH
                    sl = slice(lc * CW, (lc + 1) * CW)
                    if par == 0:
                        nc.vector.tensor_mul(out=a_packed[ti][0:64, sl],
                                             in0=pv_ps[0:64, :], in1=db[:])
                    else:
                        sh = dnp.tile([64, CW], BF16, name=f"{tag}sh",
                                      tag="sh", bufs=1)
                        nc.vector.tensor_mul(out=sh[:], in0=pv_ps[0:64, :],
                                             in1=db[:])
                        nc.sync.dma_start(out=a_packed[ti][64:128, sl], in_=sh[:])
            return a_packed

        def out_project_spill(wo_t, a_packed, bo_tiles, stage, th, tag):
            """y = wo.T @ attn for half th; bf16 spill + AllReduce trigger."""
            zin, zout = ccbuf[stage][th]
            for d in range(DT):
                zp = zpp.tile([128, SH], BF16, name=f"{tag}zp", tag="zp",
                              bufs=2)
                for c in range(NCH):
                    ps = ps_out.tile([128, CW], F32, name=f"{tag}ps",
                                     tag="psO", bufs=2)
                    for ct in range(2):
                        nc.tensor.matmul(
                            out=ps[:],
                            lhsT=wo_t[ct][:, d * 128:(d + 1) * 128],
                            rhs=a_packed[ct][:, c * CW:(c + 1) * CW],
                            start=(ct == 0), stop=(ct == 1),
                        )
                    nc.scalar.activation(
                        out=zp[:, c * CW:(c + 1) * CW], in_=ps[:],
                        func=AF.Identity, bias=bo_tiles[d][:], scale=1.0)
                dma_spread(zin[d * 128:(d + 1) * 128, :], zp[:])
            nc.gpsimd.collective_compute(
                "AllReduce", ALU.add, replica_groups=GROUPS,
                ins=[zin.opt()], outs=[zout.opt()])

        def reload_add(agg, stage, th, tag):
            """agg[:, half] += allreduced partial (bf16 -> fp32 accumulate)."""
            hs = slice(th * SH, (th + 1) * SH)
            zout = ccbuf[stage][th][1]
            for d in range(DT):
                zr = zrp.tile([128, SH], BF16, name=f"{tag}zr", tag="zr",
                              bufs=2)
                dma_spread(zr[:], zout[d * 128:(d + 1) * 128, :])
                zf = zrp.tile([128, SH], F32, name=f"{tag}zf", tag="zf",
                              bufs=2)
                nc.scalar.copy(out=zf[:], in_=zr[:])
                nc.vector.tensor_tensor(out=agg[d][:, hs], in0=zf[:],
                                        in1=agg[d][:, hs].bitcast(F32),
                                        op=ALU.add)

        def ln_half(agg, ln_idx, th, tag):
            """LayerNorm (fp32) over features for token-half th, in place."""
            hs = slice(th * SH, (th + 1) * SH)
            mu = lnp.tile([128, SH], F32, name=f"{tag}mu", tag="mu", bufs=1)
            std = lnp.tile([128, SH], F32, name=f"{tag}st", tag="st", bufs=1)
            for c in range(NCH):
                gc = th * NCH + c
                sl = slice(gc * CW, (gc + 1) * CW)
                lsl = slice(c * CW, (c + 1) * CW)
                mps = ps_proj.tile([128, CW], F32, name=f"{tag}m", tag="psP",
                                   bufs=2)
                for d in range(DT):
                    nc.tensor.matmul(out=mps[:], lhsT=ones_t[:],
                                     rhs=agg[d][:, sl],
                                     start=(d == 0), stop=(d == DT - 1))
                nc.scalar.copy(out=mu[:, lsl], in_=mps[:])
                qps = ps_proj.tile([128, CW], F32, name=f"{tag}q", tag="psP",
                                   bufs=2)
                for d in range(DT):
                    sq = lnp.tile([128, CW], F32R, name=f"{tag}sq", tag="sq",
                                  bufs=2)
                    nc.vector.tensor_mul(out=sq[:],
                                         in0=agg[d][:, sl].bitcast(F32),
                                         in1=agg[d][:, sl].bitcast(F32))
                    nc.tensor.matmul(out=qps[:], lhsT=ones_t[:], rhs=sq[:],
                                     start=(d == 0), stop=(d == DT - 1))
                musq = lnp.tile([128, CW], F32, name=f"{tag}m2", tag="m2",
                                bufs=1)
                nc.vector.tensor_mul(out=musq[:], in0=mu[:, lsl],
                                     in1=mu[:, lsl])
                var = lnp.tile([128, CW], F32, name=f"{tag}v", tag="var",
                               bufs=1)
                nc.vector.tensor_tensor(out=var[:], in0=qps[:], in1=musq[:],
                                        op=ALU.subtract)
                nc.scalar.activation(out=std[:, lsl], in_=var[:], func=AF.Sqrt,
                                     bias=eps_t[:], scale=1.0)
            nc.vector.reciprocal(out=std[:], in_=std[:])
            for d in range(DT):
                xm = lnp.tile([128, SH], F32, name=f"{tag}x", tag="xm", bufs=1)
                nc.vector.tensor_tensor(out=xm[:],
                                        in0=agg[d][:, hs].bitcast(F32),
                                        in1=mu[:], op=ALU.subtract)
                nc.vector.tensor_mul(out=xm[:], in0=xm[:], in1=std[:])
                nc.vector.tensor_scalar(
                    out=agg[d][:, hs], in0=xm[:],
                    scalar1=lng_t[ln_idx * DT + d][:],
                    scalar2=lnb_t[ln_idx * DT + d][:],
                    op0=ALU.mult, op1=ALU.add)

        # ================= pipeline =================
        # ---- P0/P1: load x + SA weights, project q/k/v ----
        agg = []
        for d in range(DT):
            t = aggp.tile([128, S], F32R, name="agg", tag="agg", bufs=8)
            dma_spread(t[:], xT[d * 128:(d + 1) * 128, :])
            agg.append(t)

        wq_t = load_wf32(wq, DC, "wqt")
        wk_t = load_wf32(wk, DC, "wkt")

        q_s = [qkp.tile([128, S], BF16, name="qs", tag="qk", bufs=4)
               for _ in range(2)]
        k_s = [qkp.tile([128, S], BF16, name="ks", tag="qk", bufs=4)
               for _ in range(2)]
        project_qk(q_s, wq_t, bq_t, agg, "qs")
        project_qk(k_s, wk_t, bk_t, agg, "ks")
        wv_t = load_wf32(wv, DCA, "wvt")
        wo_t = load_wbf(wobigp, wo, DC, D, "wob", 2)
        v_s = v_project_sa(wv_t, agg)

        # ---- P2/P3: self attention per half, spill + AR ----
        for th in range(2):
            a_h = attention_half(q_s, k_s, v_s, S, causal, th, "sa")
            out_project_spill(wo_t, a_h, bo_t, 0, th, "o1")

        # ---- P4: cross K/V from one pass over memory ----
        wkx_t = load_wbf(wsmp, wkx, D, DC, "wsm", 16)
        wvx_t = load_wbf(wsmp, wvx, D, DCA, "wsm", 16)
        wqx_t = load_wf32(wqx, DC, "wqxt")
        k_x = [qkp.tile([128, M], BF16, name="kx", tag="qk", bufs=4)
               for _ in range(2)]
        v_x = []
        for sc in range(M // CW):
            mts = []
            for d in range(DT):
                mt = memp.tile([128, CW], BF16, name="memt", tag="mem", bufs=9)
                nc.sync.dma_start(
                    out=mt[:], in_=memT[d * 128:(d + 1) * 128,
                                        sc * CW:(sc + 1) * CW])
                mts.append(mt)
            for t in range(2):
                ps = ps_proj.tile([128, CW], F32, name="kxps", tag="psP",
                                  bufs=2)
                for d in range(DT):
                    nc.tensor.matmul(
                        out=ps[:],
                        lhsT=wkx_t[d][:, t * 128:(t + 1) * 128],
                        rhs=mts[d][:],
                        start=(d == 0), stop=(d == DT - 1))
                nc.scalar.activation(
                    out=k_x[t][:, sc * CW:(sc + 1) * CW], in_=ps[:],
                    func=AF.Identity, bias=bkx_t[t][:], scale=1.0)
            for j in range(KPC):
                ps = ps_pv.tile([128, DCA], F32, name="vxps", tag="psV",
                                bufs=2)
                for d in range(DT):
                    nc.tensor.matmul(
                        out=ps[:],
                        lhsT=mts[d][:, j * 128:(j + 1) * 128],
                        rhs=wvx_t[d][:],
                        start=(d == 0), stop=(d == DT - 1))
                vt = vp.tile([128, DCA], BF16, name="vx", tag="v", bufs=18)
                nc.vector.tensor_add(out=vt[:], in0=ps[:], in1=bvxa_t[:])
                v_x.append(vt)

        # ---- P5..P7: LN1 + cross attention, per half (pipelined vs ARs) ----
        wox_t = load_wbf(wobigp, wox, DC, D, "wob", 2)
        q_x = [qkp.tile([128, S], BF16, name="qx", tag="qk", bufs=4)
               for _ in range(2)]

        def qx_project_half(th):
            for t in range(2):
                for c in range(NCH):
                    gc = th * NCH + c
                    sl = slice(gc * CW, (gc + 1) * CW)
                    ps = ps_proj.tile([128, CW], F32, name="qxps", tag="psP",
                                      bufs=2)
                    for d in range(DT):
                        nc.tensor.matmul(
                            out=ps[:],
                            lhsT=wqx_t[d][:, t * 128:(t + 1) * 128],
                            rhs=agg[d][:, sl],
                            start=(d == 0), stop=(d == DT - 1))
                    nc.scalar.activation(out=q_x[t][:, sl], in_=ps[:],
                                         func=AF.Identity, bias=bqx_t[t][:],
                                         scale=1.0)

        for th in range(2):
            reload_add(agg, 0, th, "l1")
            ln_half(agg, 0, th, "l1")
            qx_project_half(th)
            a_h = attention_half(q_x, k_x, v_x, M, False, th, "cx")
            out_project_spill(wox_t, a_h, box_t, 1, th, "o2")

        # ---- P8/P9: LN2 + FFN per half ----
        for th in range(2):
            reload_add(agg, 1, th, "l2")
            ln_half(agg, 1, th, "l2")
            h_fm = []
            # FFN1 in four f-groups so only 8 w1 tiles are live at a time
            for fc in range(4):
                w1_t = []  # [128, 256] f32r tiles: all d rows, fc col-group
                for d in range(DT):
                    t = wf32p.tile([128, 256], F32R, name="w1t", tag="w32",
                                   bufs=8)
                    DMA_ENG[d % 3].dma_start(
                        out=t[:], in_=w1[d * 128:(d + 1) * 128,
                                         fc * 256:(fc + 1) * 256])
                    w1_t.append(t)
                for f in range(fc * 2, (fc + 1) * 2):
                    ht = hp.tile([128, SH], BF16, name=f"hfm{f}", tag="hfm",
                                 bufs=8)
                    for c in range(NCH):
                        gc = th * NCH + c
                        sl = slice(gc * CW, (gc + 1) * CW)
                        ps = ps_out.tile([128, CW], F32, name="f1ps",
                                         tag="psO", bufs=2)
                        for d in range(DT):
                            nc.tensor.matmul(
                                out=ps[:],
                                lhsT=w1_t[d][:, (f % 2) * 128:
                                             (f % 2 + 1) * 128],
                                rhs=agg[d][:, sl],
                                start=(d == 0), stop=(d == DT - 1))
                        nc.scalar.activation(out=ht[:, c * CW:(c + 1) * CW],
                                             in_=ps[:], func=AF.Relu,
                                             bias=b1_t[f][:], scale=1.0)
                    h_fm.append(ht)
            # FFN2 in four d-groups so only 8 w2 tiles are live at a time
            zin, zout = ccbuf[2][th]
            for dc2 in range(4):
                w2_t = []  # [128, 256] bf16 tiles: all f rows, dc2 col-group
                for f in range(NF):
                    t = w2p.tile([128, 256], BF16, name="w2t", tag="w2",
                                 bufs=8)
                    DMA_ENG[f % 3].dma_start(
                        out=t[:], in_=w2[f * 128:(f + 1) * 128,
                                         dc2 * 256:(dc2 + 1) * 256])
                    w2_t.append(t)
                for d in range(dc2 * 2, (dc2 + 1) * 2):
                    zp = zpp.tile([128, SH], BF16, name="f2zp", tag="zp",
                                  bufs=2)
                    for c in range(NCH):
                        ps = ps_out.tile([128, CW], F32, name="f2ps",
                                         tag="psO", bufs=2)
                        for f in range(NF):
                            nc.tensor.matmul(
                                out=ps[:],
                                lhsT=w2_t[f][:, (d % 2) * 128:
                                             (d % 2 + 1) * 128],
                                rhs=h_fm[f][:, c * CW:(c + 1) * CW],
                                start=(f == 0), stop=(f == NF - 1))
                        nc.scalar.activation(
                            out=zp[:, c * CW:(c + 1) * CW], in_=ps[:],
                            func=AF.Identity, bias=b2_t[d][:], scale=1.0)
                    dma_spread(zin[d * 128:(d + 1) * 128, :], zp[:])
            nc.gpsimd.collective_compute(
                "AllReduce", ALU.add, replica_groups=GROUPS,
                ins=[zin.opt()], outs=[zout.opt()])

        # ---- P10/P11: LN3 + output ----
        for th in range(2):
            reload_add(agg, 2, th, "l3")
            ln_half(agg, 2, th, "l3")
            hs = slice(th * SH, (th + 1) * SH)
            for d in range(DT):
                dma_spread(outT[d * 128:(d + 1) * 128, hs], agg[d][:, hs])

    nc.finalize()
    return nc


def _get_nc(S, M, causal):
    key = (S, M, causal)
    if key not in _nc_cache:
        _nc_cache[key] = _build(S, M, causal)
    return _nc_cache[key]


def _prep_inputs(c, S, M, tgt, memory, Wqkv, bqkv, Wo_sa, bo_sa, Wq, bq, Wk, bk,
                 Wv, bv, Wo_cx, bo_cx, W1, b1, W2, b2, g_mha, bn_mha, g_crx,
                 bn_crx, g_ffn, bn_ffn):
    r, b = c % TP, c // TP
    hsl = slice(r * DC, (r + 1) * DC)
    fsl = slice(r * FFC, (r + 1) * FFC)
    f32 = np.float32
    bf = ml_dtypes.bfloat16

    def aug_v(wv_c, bv_c):
        wva = np.zeros((D, DCA), f32)
        bva = np.zeros((1, DCA), f32)
        for h in range(HL):
            wva[:, h * 65:h * 65 + 64] = wv_c[:, h * 64:(h + 1) * 64]
            bva[0, h * 65:h * 65 + 64] = bv_c[h * 64:(h + 1) * 64]
            bva[0, h * 65 + 64] = 1.0
        return wva, bva

    scale = np.float32(1.0 / np.sqrt(HD))
    # Wqkv columns are per-head interleaved: head g = cols g*192 + [q64|k64|v64]
    wqkv_h = Wqkv.reshape(D, H, 3 * HD)
    bqkv_h = bqkv.reshape(H, 3 * HD)
    gh = slice(r * HL, (r + 1) * HL)  # this rank's global heads
    wq_sa = wqkv_h[:, gh, 0:HD].reshape(D, DC) * scale
    wk_sa = wqkv_h[:, gh, HD:2 * HD].reshape(D, DC)
    wv_sa = wqkv_h[:, gh, 2 * HD:3 * HD].reshape(D, DC)
    bq_sa = bqkv_h[gh, 0:HD].reshape(DC) * scale
    bk_sa = bqkv_h[gh, HD:2 * HD].reshape(DC)
    bv_sa = bqkv_h[gh, 2 * HD:3 * HD].reshape(DC)
    wva_sa, bva_sa = aug_v(wv_sa, bv_sa)
    wvx_c, bvx_c = aug_v(Wv[:, hsl], bv[hsl])
    rank0 = np.float32(1.0 if r == 0 else 0.0)
    return {
        "xT": np.ascontiguousarray(tgt[b].T, f32),
        "memT": np.ascontiguousarray(memory[b].T).astype(bf),
        "wq": np.ascontiguousarray(wq_sa, f32),
        "wk": np.ascontiguousarray(wk_sa, f32),
        "wv": wva_sa,
        "bq": np.ascontiguousarray(bq_sa.reshape(DC, 1), f32),
        "bk": np.ascontiguousarray(bk_sa.reshape(DC, 1), f32),
        "bva": bva_sa,
        "wo": np.ascontiguousarray(Wo_sa[hsl, :]).astype(bf),
        "bo": np.ascontiguousarray((bo_sa * rank0).reshape(D, 1), f32),
        "wqx": np.ascontiguousarray(Wq[:, hsl] * scale, f32),
        "wkx": np.ascontiguousarray(Wk[:, hsl]).astype(bf),
        "wvx": wvx_c.astype(bf),
        "bqx": np.ascontiguousarray((bq[hsl] * scale).reshape(DC, 1), f32),
        "bkx": np.ascontiguousarray(bk[hsl].reshape(DC, 1), f32),
        "bvxa": bvx_c,
        "wox": np.ascontiguousarray(Wo_cx[hsl, :]).astype(bf),
        "box": np.ascontiguousarray((bo_cx * rank0).reshape(D, 1), f32),
        "w1": np.ascontiguousarray(W1[:, fsl], f32),
        "b1": np.ascontiguousarray(b1[fsl].reshape(FFC, 1), f32),
        "w2": np.ascontiguousarray(W2[fsl, :]).astype(bf),
        "b2": np.ascontiguousarray((b2 * rank0).reshape(D, 1), f32),
        "lng": np.ascontiguousarray(
            np.concatenate([g_mha, g_crx, g_ffn]).reshape(3 * D, 1), f32),
        "lnb": np.ascontiguousarray(
            np.concatenate([bn_mha, bn_crx, bn_ffn]).reshape(3 * D, 1), f32),
        "ones": np.full((128, 128), 1.0 / D, f32),
    }


def kernel(**inputs):
    tgt = np.asarray(inputs["tgt"], np.float32)
    memory = np.asarray(inputs["memory"], np.float32)
    mask = np.asarray(inputs["tgt_mask"])
    S, M = tgt.shape[1], memory.shape[1]

    if mask.any():
        expect = np.triu(np.ones((S, S), bool), 1)
        if not np.array_equal(mask, expect):
            raise NotImplementedError("only causal or empty tgt_mask supported")
        causal = True
    else:
        causal = False

    nc = _get_nc(S, M, causal)
    args = {k: np.asarray(v, np.float32) for k, v in inputs.items()
            if k not in ("tgt", "memory", "tgt_mask")}
    in_maps = [_prep_inputs(c, S, M, tgt, memory, **args) for c in range(NCORES)]

    trace = bool(int(os.environ.get("BASS_KERNEL_TRACE", "0")))
    res = run_bass_kernel_spmd(nc, in_maps, list(range(NCORES)), trace=trace)
    if trace:
        kernel.last_exec_time_ns = res.exec_time_ns
    out = np.stack([
        np.ascontiguousarray(res.results[0]["outT"].T),
        np.ascontiguousarray(res.results[TP]["outT"].T),
    ])
    return out.astype(np.float32)


# revision 40
# speedup vs baseline: 1.2576x; 1.0119x over previous
"""Trainium2 Bass kernel for nn_DecoderBlock (self-attn + cross-attn + FFN, post-LN).

Sharding: data-parallel over batch (2 groups of 4 cores), tensor-parallel over
heads / FFN hidden dim within each group. Three AllReduces per group, chunked
into token-halves and software-pipelined so each AllReduce overlaps the other
half's compute (including the next stage's work for the already-reduced half).

Precision: fp32 residual stream + LayerNorm; bf16 weights/attention/FFN-hidden
and bf16 AllReduce payloads (validated ~2e-3 rel err vs fp64 reference).

All on-device activations are feature-major ([features on partitions, tokens
on free axis]). The host pre-transposes tgt/memory and post-transposes out.
"""

import os
import sys

sys.path.insert(0, "/opt/trn_rl_repo")

from contextlib import ExitStack

import numpy as np
import ml_dtypes

import concourse.bacc as bacc
import concourse.tile as tile
from concourse import mybir
from concourse.bass_utils import run_bass_kernel_spmd

F32R = mybir.dt.float32r
F32 = mybir.dt.float32
BF16 = mybir.dt.bfloat16
AF = mybir.ActivationFunctionType
ALU = mybir.AluOpType

B = 2
D = 1024
H = 16
HD = 64
FF = 4 * D
NCORES = 8
TP = 4
HL = H // TP          # 4 local heads
DC = HL * HD          # 256 local q/k/v features
DCA = HL * (HD + 1)   # 260: V augmented with a ones column per head
FFC = FF // TP        # 1024 local ffn features
GROUPS = [[0, 1, 2, 3], [4, 5, 6, 7]]
DT = D // 128         # 8 feature partition-tiles
NF = FFC // 128       # 8 ffn partition-tiles

_nc_cache = {}


def _build(S, M, causal):
    nc = bacc.Bacc(None, target_bir_lowering=False, num_devices=NCORES)

    SH = S // 2           # tokens per pipeline half
    CW = 512              # chunk width
    NCH = SH // CW        # chunks per half
    KPC = CW // 128       # key tiles per chunk width

    # ---- DRAM parameters ----
    dp = nc.declare_dram_parameter
    xT = dp("xT", [D, S], F32R, isOutput=False)
    memT = dp("memT", [D, M], BF16, isOutput=False)
    wq = dp("wq", [D, DC], F32R, isOutput=False)
    wk = dp("wk", [D, DC], F32R, isOutput=False)
    wv = dp("wv", [D, DCA], F32R, isOutput=False)
    bva = dp("bva", [1, DCA], F32, isOutput=False)
    wo = dp("wo", [DC, D], BF16, isOutput=False)
    wqx = dp("wqx", [D, DC], F32R, isOutput=False)
    wkx = dp("wkx", [D, DC], BF16, isOutput=False)
    wvx = dp("wvx", [D, DCA], BF16, isOutput=False)
    bvxa = dp("bvxa", [1, DCA], F32, isOutput=False)
    wox = dp("wox", [DC, D], BF16, isOutput=False)
    w1 = dp("w1", [D, FFC], F32R, isOutput=False)
    w2 = dp("w2", [FFC, D], BF16, isOutput=False)
    # packed [128,1] bias columns: bq(2) bk(2) bqx(2) bkx(2) bo(8) box(8)
    # b1(8) b2(8) lng(24) lnb(24)
    biases = dp("biases", [128, 88], F32, isOutput=False)
    ones = dp("ones", [128, 128], F32R, isOutput=False)
    outT = dp("outT", [D, S], F32R, isOutput=True)

    with tile.TileContext(nc) as tc, ExitStack() as st:
        ep = st.enter_context
        constp = ep(tc.tile_pool(name="const", bufs=1))
        aggp = ep(tc.tile_pool(name="agg", bufs=8))
        wf32p = ep(tc.tile_pool(name="wf32", bufs=8))
        wobigp = ep(tc.tile_pool(name="wobig", bufs=2))
        wsmp = ep(tc.tile_pool(name="wsm", bufs=16))
        w2p = ep(tc.tile_pool(name="w2p", bufs=8))
        qkp = ep(tc.tile_pool(name="qk", bufs=4))
        vp = ep(tc.tile_pool(name="vp", bufs=17))
        memp = ep(tc.tile_pool(name="memp", bufs=8))
        hp = ep(tc.tile_pool(name="hp", bufs=8))
        app = ep(tc.tile_pool(name="ap", bufs=3))
        ppool = ep(tc.tile_pool(name="pp", bufs=4))
        zpp = ep(tc.tile_pool(name="zpp", bufs=2))
        zrp = ep(tc.tile_pool(name="zrp", bufs=2))
        lnp = ep(tc.tile_pool(name="lnp", bufs=1))
        dnp = ep(tc.tile_pool(name="dnp", bufs=2))
        dramp = ep(tc.tile_pool(name="dram", bufs=1, space="DRAM"))
        ps_proj = ep(tc.tile_pool(name="psP", bufs=2, space="PSUM"))
        ps_sc = ep(tc.tile_pool(name="psS", bufs=2, space="PSUM"))
        ps_pv = ep(tc.tile_pool(name="psV", bufs=2, space="PSUM"))
        ps_out = ep(tc.tile_pool(name="psO", bufs=2, space="PSUM"))

        # After the first collective trigger, gpsimd carries ONLY triggers
        # (they execute synchronously and head-of-line block anything queued
        # behind them). Prologue DMAs may still use it.
        DMA_ENG = [nc.sync, nc.scalar]
        DMA_ENG3 = [nc.sync, nc.scalar, nc.gpsimd]

        def dma_spread(out_ap, in_ap, nsplit=4, engines=None):
            engines = engines or DMA_ENG
            p = out_ap.shape[0]
            step = p // nsplit
            for i in range(nsplit):
                sl = slice(i * step, (i + 1) * step if i < nsplit - 1 else p)
                engines[i % len(engines)].dma_start(out=out_ap[sl],
                                                    in_=in_ap[sl])

        # ---- constants ----
        ones_t = constp.tile([128, 128], F32R, name="ones_t")
        nc.scalar.dma_start(out=ones_t[:], in_=ones[:, :])
        eps_t = constp.tile([128, 1], F32, name="eps_t")
        nc.vector.memset(eps_t[:], 1e-5)

        ball = constp.tile([128, 88], F32, name="ball")
        nc.sync.dma_start(out=ball[:], in_=biases[:, :])

        def bias_tiles(col0, n):
            return [ball[:, col0 + i:col0 + i + 1] for i in range(n)]

        bq_t = bias_tiles(0, 2)
        bk_t = bias_tiles(2, 2)
        bqx_t = bias_tiles(4, 2)
        bkx_t = bias_tiles(6, 2)
        bo_t = bias_tiles(8, DT)
        box_t = bias_tiles(16, DT)
        b1_t = bias_tiles(24, NF)
        b2_t = bias_tiles(32, DT)
        lng_t = bias_tiles(40, 3 * DT)
        lnb_t = bias_tiles(64, 3 * DT)
        bva_t = constp.tile([128, DCA], F32, name="bva_t")
        nc.scalar.dma_start(out=bva_t[:], in_=bva[:, :].to_broadcast([128, DCA]))
        bvxa_t = constp.tile([128, DCA], F32, name="bvxa_t")
        nc.scalar.dma_start(out=bvxa_t[:],
                            in_=bvxa[:, :].to_broadcast([128, DCA]))

        # ones row at partition 64 for the K=1 denominator-broadcast matmul
        ones65 = constp.tile([65, 64], F32R, name="ones65")
        nc.vector.memset(ones65[:].bitcast(F32), 1.0)

        # 4 static causal mask tiles (prologue gpsimd use is safe: no
        # collective has been triggered yet). mask_j[k, q] = (q - 128j >= k).
        mask_t = []
        if causal:
            mones = constp.tile([128, CW], BF16, name="mones")
            nc.vector.memset(mones[:], 1.0)
            for j in range(KPC):
                mt = constp.tile([128, CW], BF16, name=f"mask{j}")
                nc.gpsimd.affine_select(
                    out=mt[:], in_=mones[:], pattern=[[1, CW]],
                    compare_op=ALU.is_ge, fill=0.0,
                    base=-128 * j, channel_multiplier=-1)
                mask_t.append(mt)

        # ---- DRAM bounce buffers: unique tag per tensor (no aliasing) ----
        ccbuf = [[(dramp.tile([D, SH], BF16, name=f"zin{s_}{h_}",
                              tag=f"zin{s_}{h_}"),
                   dramp.tile([D, SH], BF16, name=f"zout{s_}{h_}",
                              tag=f"zout{s_}{h_}"))
                  for h_ in range(2)] for s_ in range(3)]

        # ================= helpers =================
        def load_wf32(src, cols, tag="w32", engines=None):
            """8 [128, cols] f32r tiles from the shared f32 weight ring."""
            engines = engines or DMA_ENG
            ts = []
            for d in range(DT):
                t = wf32p.tile([128, cols], F32R, name=tag, tag="w32", bufs=8)
                engines[d % len(engines)].dma_start(
                    out=t[:], in_=src[d * 128:(d + 1) * 128, :])
                ts.append(t)
            return ts

        def load_wbf(pool, src, rows, cols, tag, bufs, engines=None):
            """rows//128 [128, cols] bf16 tiles from a bf16 weight ring."""
            engines = engines or DMA_ENG
            ts = []
            for d in range(rows // 128):
                t = pool.tile([128, cols], BF16, name=tag, tag=tag, bufs=bufs)
                engines[d % len(engines)].dma_start(
                    out=t[:], in_=src[d * 128:(d + 1) * 128, :])
                ts.append(t)
            return ts

        def project_qk(qt_pair, w_tiles, b_tiles, src_tiles, tag):
            """Full-S q/k projection into 2 packed [128, S] bf16 tiles."""
            for t in range(2):
                for sc in range(S // CW):
                    sl = slice(sc * CW, (sc + 1) * CW)
                    ps = ps_proj.tile([128, CW], F32, name=f"{tag}ps",
                                      tag="psP", bufs=2)
                    for d in range(DT):
                        nc.tensor.matmul(
                            out=ps[:],
                            lhsT=w_tiles[d][:, t * 128:(t + 1) * 128],
                            rhs=src_tiles[d][:, sl],
                            start=(d == 0), stop=(d == DT - 1),
                        )
                    nc.vector.tensor_scalar_add(qt_pair[t][:, sl], ps[:],
                                                b_tiles[t][:])

        def v_project_sa(w_tiles, src_tiles):
            """Self-attn V: token-major [128, DCA] bf16 tiles, one per 128 toks."""
            vs = []
            for s_t in range(S // 128):
                ps = ps_pv.tile([128, DCA], F32, name="vps", tag="psV", bufs=2)
                for d in range(DT):
                    nc.tensor.matmul(
                        out=ps[:],
                        lhsT=src_tiles[d][:, s_t * 128:(s_t + 1) * 128],
                        rhs=w_tiles[d][:],
                        start=(d == 0), stop=(d == DT - 1))
                vt = vp.tile([128, DCA], BF16, name="vs", tag="v", bufs=18)
                nc.vector.tensor_add(out=vt[:], in0=ps[:], in1=bva_t[:])
                vs.append(vt)
            return vs

        def attention_half(q_pair, k_pair, v_tiles, n_keys, use_mask, th, tag):
            """Attention for token-half th -> 2 packed [128, SH] bf16 tiles."""
            a_packed = [app.tile([128, SH], BF16, name=f"{tag}{t}", tag="attn",
                                 bufs=3) for t in range(2)]
            kt_total = n_keys // 128
            for qc in range(th * NCH, (th + 1) * NCH):
                for h in range(HL):
                    par, ti = h % 2, h // 2
                    kts = range(min(kt_total, KPC * (qc + 1)) if use_mask
                                else kt_total)
                    n_kt = len(kts)
                    pv_ps = ps_pv.tile([65, CW], F32, name=f"{tag}pv",
                                       tag="psV", bufs=2)
                    for i, kt in enumerate(kts):
                        s_ps = ps_sc.tile([128, CW], F32, name=f"{tag}s",
                                          tag="psS", bufs=2)
                        nc.tensor.matmul(
                            out=s_ps[:],
                            lhsT=k_pair[ti][par * 64:(par + 1) * 64,
                                            kt * 128:(kt + 1) * 128],
                            rhs=q_pair[ti][par * 64:(par + 1) * 64,
                                           qc * CW:(qc + 1) * CW],
                            start=True, stop=True,
                        )
                        p_t = ppool.tile([128, CW], BF16, name=f"{tag}p",
                                         tag="p", bufs=3)
                        nc.scalar.activation(out=p_t[:], in_=s_ps[:], func=AF.Exp)
                        if use_mask and kt >= KPC * qc:
                            p_m = ppool.tile([128, CW], BF16, name=f"{tag}pm",
                                             tag="p", bufs=3)
                            nc.vector.tensor_mul(out=p_m[:], in0=p_t[:],
                                                 in1=mask_t[kt - KPC * qc][:])
                            p_use = p_m
                        else:
                            p_use = p_t
                        nc.tensor.matmul(
                            out=pv_ps[:],
                            lhsT=v_tiles[kt][:, h * 65:(h + 1) * 65],
                            rhs=p_use[:],
                            start=(i == 0), stop=(i == n_kt - 1),
                        )
                    # softmax denominator: reciprocal on row 64, then
                    # broadcast down 64 partitions via a K=1 matmul.
                    dsc = dnp.tile([65, CW], F32R, name=f"{tag}dsc", tag="dsc",
                                   bufs=2)
                    nc.scalar.activation(out=dsc[64:65, :],
                                         in_=pv_ps[64:65, :],
                                         func=AF.Identity)
                    with nc.allow_low_precision(
                            reason="softmax denom reciprocal fed to f32r "
                                   "broadcast matmul; f32r rounding is ~1e-5"):
                        nc.vector.reciprocal(out=dsc[64:65, :],
                                             in_=dsc[64:65, :].bitcast(F32))
                    db_ps = ps_pv.tile([64, CW], F32, name=f"{tag}dbp",
                                       tag="psV", bufs=2)
                    nc.tensor.matmul(out=db_ps[:], lhsT=ones65[64:65, :],
                                     rhs=dsc[64:65, :], start=True, stop=True)
                    db = dnp.tile([64, CW], F32, name=f"{tag}db", tag="db",
                                  bufs=2)
                    nc.vector.tensor_scalar_add(db[:], db_ps[:], 0.0)
                    lc = qc - th * NCH
                    sl = slice(lc * CW, (lc + 1) * CW)
                    if par == 0:
                        nc.vector.tensor_mul(out=a_packed[ti][0:64, sl],
                                             in0=pv_ps[0:64, :], in1=db[:])
                    else:
                        sh = dnp.tile([64, CW], BF16, name=f"{tag}sh",
                                      tag="sh", bufs=1)
                        nc.vector.tensor_mul(out=sh[:], in0=pv_ps[0:64, :],
                                             in1=db[:])
                        nc.sync.dma_start(out=a_packed[ti][64:128, sl], in_=sh[:])
            return a_packed

        def out_project_spill(wo_t, a_packed, bo_tiles, stage, th, tag):
            """y = wo.T @ attn for half th; bf16 spill + AllReduce trigger."""
            zin, zout = ccbuf[stage][th]
            for d in range(DT):
                zp = zpp.tile([128, SH], BF16, name=f"{tag}zp", tag="zp",
                              bufs=2)
                for c in range(NCH):
                    ps = ps_out.tile([128, CW], F32, name=f"{tag}ps",
                                     tag="psO", bufs=2)
                    for ct in range(2):
                        nc.tensor.matmul(
                            out=ps[:],
                            lhsT=wo_t[ct][:, d * 128:(d + 1) * 128],
                            rhs=a_packed[ct][:, c * CW:(c + 1) * CW],
                            start=(ct == 0), stop=(ct == 1),
                        )
                    nc.scalar.activation(
                        out=zp[:, c * CW:(c + 1) * CW], in_=ps[:],
                        func=AF.Identity, bias=bo_tiles[d][:], scale=1.0)
                dma_spread(zin[d * 128:(d + 1) * 128, :], zp[:])
            nc.gpsimd.collective_compute(
                "AllReduce", ALU.add, replica_groups=GROUPS,
                ins=[zin.opt()], outs=[zout.opt()])

        def reload_add(agg, stage, th, tag):
            """agg[:, half] += allreduced partial (bf16 -> fp32 accumulate)."""
            hs = slice(th * SH, (th + 1) * SH)
            zout = ccbuf[stage][th][1]
            for d in range(DT):
                zr = zrp.tile([128, SH], BF16, name=f"{tag}zr", tag="zr",
                              bufs=2)
                dma_spread(zr[:], zout[d * 128:(d + 1) * 128, :])
                zf = zrp.tile([128, SH], F32, name=f"{tag}zf", tag="zf",
                              bufs=2)
                nc.scalar.copy(out=zf[:], in_=zr[:])
                nc.vector.tensor_tensor(out=agg[d][:, hs], in0=zf[:],
                                        in1=agg[d][:, hs].bitcast(F32),
                                        op=ALU.add)

        def ln_half(agg, ln_idx, th, tag):
            """LayerNorm (fp32) over features for token-half th, in place."""
            hs = slice(th * SH, (th + 1) * SH)
            mu = lnp.tile([128, SH], F32, name=f"{tag}mu", tag="mu", bufs=1)
            std = lnp.tile([128, SH], F32, name=f"{tag}st", tag="st", bufs=1)
            for c in range(NCH):
                gc = th * NCH + c
                sl = slice(gc * CW, (gc + 1) * CW)
                lsl = slice(c * CW, (c + 1) * CW)
                mps = ps_proj.tile([128, CW], F32, name=f"{tag}m", tag="psP",
                                   bufs=2)
                for d in range(DT):
                    nc.tensor.matmul(out=mps[:], lhsT=ones_t[:],
                                     rhs=agg[d][:, sl],
                                     start=(d == 0), stop=(d == DT - 1))
                nc.scalar.copy(out=mu[:, lsl], in_=mps[:])
                qps = ps_proj.tile([128, CW], F32, name=f"{tag}q", tag="psP",
                                   bufs=2)
                for d in range(DT):
                    sq = lnp.tile([128, CW], F32R, name=f"{tag}sq", tag="sq",
                                  bufs=2)
                    nc.vector.tensor_mul(out=sq[:],
                                         in0=agg[d][:, sl].bitcast(F32),
                                         in1=agg[d][:, sl].bitcast(F32))
                    nc.tensor.matmul(out=qps[:], lhsT=ones_t[:], rhs=sq[:],
                                     start=(d == 0), stop=(d == DT - 1))
                musq = lnp.tile([128, CW], F32, name=f"{tag}m2", tag="m2",
                                bufs=1)
                nc.vector.tensor_mul(out=musq[:], in0=mu[:, lsl],
                                     in1=mu[:, lsl])
                var = lnp.tile([128, CW], F32, name=f"{tag}v", tag="var",
                               bufs=1)
                nc.vector.tensor_tensor(out=var[:], in0=qps[:], in1=musq[:],
                                        op=ALU.subtract)
                nc.scalar.activation(out=std[:, lsl], in_=var[:], func=AF.Sqrt,
                                     bias=eps_t[:], scale=1.0)
            nc.vector.reciprocal(out=std[:], in_=std[:])
            for d in range(DT):
                xm = lnp.tile([128, SH], F32, name=f"{tag}x", tag="xm", bufs=1)
                nc.vector.tensor_tensor(out=xm[:],
                                        in0=agg[d][:, hs].bitcast(F32),
                                        in1=mu[:], op=ALU.subtract)
                nc.vector.tensor_mul(out=xm[:], in0=xm[:], in1=std[:])
                nc.vector.tensor_scalar(
                    out=agg[d][:, hs], in0=xm[:],
                    scalar1=lng_t[ln_idx * DT + d][:],
                    scalar2=lnb_t[ln_idx * DT + d][:],
                    op0=ALU.mult, op1=ALU.add)

        # ================= pipeline =================
        # ---- P0/P1: load x + SA weights, project q/k/v ----
        agg = []
        for d in range(DT):
            t = aggp.tile([128, S], F32R, name="agg", tag="agg", bufs=8)
            dma_spread(t[:], xT[d * 128:(d + 1) * 128, :], nsplit=3,
                       engines=DMA_ENG3)
            agg.append(t)

        wq_t = load_wf32(wq, DC, "wqt", DMA_ENG3)
        wk_t = load_wf32(wk, DC, "wkt", DMA_ENG3)

        q_s = [qkp.tile([128, S], BF16, name="qs", tag="qk", bufs=4)
               for _ in range(2)]
        k_s = [qkp.tile([128, S], BF16, name="ks", tag="qk", bufs=4)
               for _ in range(2)]
        project_qk(q_s, wq_t, bq_t, agg, "qs")
        project_qk(k_s, wk_t, bk_t, agg, "ks")
        wv_t = load_wf32(wv, DCA, "wvt", DMA_ENG3)
        wo_t = load_wbf(wobigp, wo, DC, D, "wob", 2, DMA_ENG3)
        v_s = v_project_sa(wv_t, agg)

        # ---- P2/P3: self attention per half, spill + AR ----
        for th in range(2):
            a_h = attention_half(q_s, k_s, v_s, S, causal, th, "sa")
            out_project_spill(wo_t, a_h, bo_t, 0, th, "o1")

        # ---- P4: cross K/V from one pass over memory ----
        wkx_t = load_wbf(wsmp, wkx, D, DC, "wsm", 16)
        wvx_t = load_wbf(wsmp, wvx, D, DCA, "wsm", 16)
        wqx_t = load_wf32(wqx, DC, "wqxt")
        k_x = [qkp.tile([128, M], BF16, name="kx", tag="qk", bufs=4)
               for _ in range(2)]
        v_x = []
        for sc in range(M // CW):
            mts = []
            for d in range(DT):
                mt = memp.tile([128, CW], BF16, name="memt", tag="mem", bufs=9)
                nc.sync.dma_start(
                    out=mt[:], in_=memT[d * 128:(d + 1) * 128,
                                        sc * CW:(sc + 1) * CW])
                mts.append(mt)
            for t in range(2):
                ps = ps_proj.tile([128, CW], F32, name="kxps", tag="psP",
                                  bufs=2)
                for d in range(DT):
                    nc.tensor.matmul(
                        out=ps[:],
                        lhsT=wkx_t[d][:, t * 128:(t + 1) * 128],
                        rhs=mts[d][:],
                        start=(d == 0), stop=(d == DT - 1))
                nc.scalar.activation(
                    out=k_x[t][:, sc * CW:(sc + 1) * CW], in_=ps[:],
                    func=AF.Identity, bias=bkx_t[t][:], scale=1.0)
            for j in range(KPC):
                ps = ps_pv.tile([128, DCA], F32, name="vxps", tag="psV",
                                bufs=2)
                for d in range(DT):
                    nc.tensor.matmul(
                        out=ps[:],
                        lhsT=mts[d][:, j * 128:(j + 1) * 128],
                        rhs=wvx_t[d][:],
                        start=(d == 0), stop=(d == DT - 1))
                vt = vp.tile([128, DCA], BF16, name="vx", tag="v", bufs=18)
                nc.vector.tensor_add(out=vt[:], in0=ps[:], in1=bvxa_t[:])
                v_x.append(vt)

        # ---- P5..P7: LN1 + cross attention, per half (pipelined vs ARs) ----
        wox_t = load_wbf(wobigp, wox, DC, D, "wob", 2)
        q_x = [qkp.tile([128, S], BF16, name="qx", tag="qk", bufs=4)
               for _ in range(2)]

        def qx_project_half(th):
            for t in range(2):
                for c in range(NCH):
                    gc = th * NCH + c
                    sl = slice(gc * CW, (gc + 1) * CW)
                    ps = ps_proj.tile([128, CW], F32, name="qxps", tag="psP",
                                      bufs=2)
                    for d in range(DT):
                        nc.tensor.matmul(
                            out=ps[:],
                            lhsT=wqx_t[d][:, t * 128:(t + 1) * 128],
                            rhs=agg[d][:, sl],
                            start=(d == 0), stop=(d == DT - 1))
                    nc.vector.tensor_scalar_add(q_x[t][:, sl], ps[:],
                                                bqx_t[t][:])

        for th in range(2):
            reload_add(agg, 0, th, "l1")
            ln_half(agg, 0, th, "l1")
            qx_project_half(th)
            a_h = attention_half(q_x, k_x, v_x, M, False, th, "cx")
            out_project_spill(wox_t, a_h, box_t, 1, th, "o2")

        # ---- P8/P9: LN2 + FFN per half ----
        for th in range(2):
            reload_add(agg, 1, th, "l2")
            ln_half(agg, 1, th, "l2")
            h_fm = []
            # FFN1 in four f-groups so only 8 w1 tiles are live at a time
            for fc in range(4):
                w1_t = []  # [128, 256] f32r tiles: all d rows, fc col-group
                for d in range(DT):
                    t = wf32p.tile([128, 256], F32R, name="w1t", tag="w32",
                                   bufs=8)
                    DMA_ENG[d % 2].dma_start(
                        out=t[:], in_=w1[d * 128:(d + 1) * 128,
                                         fc * 256:(fc + 1) * 256])
                    w1_t.append(t)
                for f in range(fc * 2, (fc + 1) * 2):
                    ht = hp.tile([128, SH], BF16, name=f"hfm{f}", tag="hfm",
                                 bufs=8)
                    for c in range(NCH):
                        gc = th * NCH + c
                        sl = slice(gc * CW, (gc + 1) * CW)
                        ps = ps_out.tile([128, CW], F32, name="f1ps",
                                         tag="psO", bufs=2)
                        for d in range(DT):
                            nc.tensor.matmul(
                                out=ps[:],
                                lhsT=w1_t[d][:, (f % 2) * 128:
                                             (f % 2 + 1) * 128],
                                rhs=agg[d][:, sl],
                                start=(d == 0), stop=(d == DT - 1))
                        nc.scalar.activation(out=ht[:, c * CW:(c + 1) * CW],
                                             in_=ps[:], func=AF.Relu,
                                             bias=b1_t[f][:], scale=1.0)
                    h_fm.append(ht)
            # FFN2 in four d-groups so only 8 w2 tiles are live at a time
            zin, zout = ccbuf[2][th]
            for dc2 in range(4):
                w2_t = []  # [128, 256] bf16 tiles: all f rows, dc2 col-group
                for f in range(NF):
                    t = w2p.tile([128, 256], BF16, name="w2t", tag="w2",
                                 bufs=8)
                    DMA_ENG[f % 2].dma_start(
                        out=t[:], in_=w2[f * 128:(f + 1) * 128,
                                         dc2 * 256:(dc2 + 1) * 256])
                    w2_t.append(t)
                for d in range(dc2 * 2, (dc2 + 1) * 2):
                    zp = zpp.tile([128, SH], BF16, name="f2zp", tag="zp",
                                  bufs=2)
                    for c in range(NCH):
                        ps = ps_out.tile([128, CW], F32, name="f2ps",
                                         tag="psO", bufs=2)
                        for f in range(NF):
                            nc.tensor.matmul(
                                out=ps[:],
                                lhsT=w2_t[f][:, (d % 2) * 128:
                                             (d % 2 + 1) * 128],
                                rhs=h_fm[f][:, c * CW:(c + 1) * CW],
                                start=(f == 0), stop=(f == NF - 1))
                        nc.scalar.activation(
                            out=zp[:, c * CW:(c + 1) * CW], in_=ps[:],
                            func=AF.Identity, bias=b2_t[d][:], scale=1.0)
                    dma_spread(zin[d * 128:(d + 1) * 128, :], zp[:])
            nc.gpsimd.collective_compute(
                "AllReduce", ALU.add, replica_groups=GROUPS,
                ins=[zin.opt()], outs=[zout.opt()])

        # ---- P10/P11: LN3 + output ----
        for th in range(2):
            reload_add(agg, 2, th, "l3")
            ln_half(agg, 2, th, "l3")
            hs = slice(th * SH, (th + 1) * SH)
            for d in range(DT):
                dma_spread(outT[d * 128:(d + 1) * 128, hs], agg[d][:, hs])

    nc.finalize()
    return nc


def _get_nc(S, M, causal):
    key = (S, M, causal)
    if key not in _nc_cache:
        _nc_cache[key] = _build(S, M, causal)
    return _nc_cache[key]


def _prep_inputs(c, S, M, tgt, memory, Wqkv, bqkv, Wo_sa, bo_sa, Wq, bq, Wk, bk,
                 Wv, bv, Wo_cx, bo_cx, W1, b1, W2, b2, g_mha, bn_mha, g_crx,
                 bn_crx, g_ffn, bn_ffn):
    r, b = c % TP, c // TP
    hsl = slice(r * DC, (r + 1) * DC)
    fsl = slice(r * FFC, (r + 1) * FFC)
    f32 = np.float32
    bf = ml_dtypes.bfloat16

    def aug_v(wv_c, bv_c):
        wva = np.zeros((D, DCA), f32)
        bva = np.zeros((1, DCA), f32)
        for h in range(HL):
            wva[:, h * 65:h * 65 + 64] = wv_c[:, h * 64:(h + 1) * 64]
            bva[0, h * 65:h * 65 + 64] = bv_c[h * 64:(h + 1) * 64]
            bva[0, h * 65 + 64] = 1.0
        return wva, bva

    def pack_biases(cols):
        out = np.zeros((128, 88), f32)
        i = 0
        for v in cols:
            v = np.asarray(v, f32).reshape(-1)
            n = len(v) // 128
            for j in range(n):
                out[:, i] = v[j * 128:(j + 1) * 128]
                i += 1
        assert i == 88
        return out

    scale = np.float32(1.0 / np.sqrt(HD))
    # Wqkv columns are per-head interleaved: head g = cols g*192 + [q64|k64|v64]
    wqkv_h = Wqkv.reshape(D, H, 3 * HD)
    bqkv_h = bqkv.reshape(H, 3 * HD)
    gh = slice(r * HL, (r + 1) * HL)  # this rank's global heads
    wq_sa = wqkv_h[:, gh, 0:HD].reshape(D, DC) * scale
    wk_sa = wqkv_h[:, gh, HD:2 * HD].reshape(D, DC)
    wv_sa = wqkv_h[:, gh, 2 * HD:3 * HD].reshape(D, DC)
    bq_sa = bqkv_h[gh, 0:HD].reshape(DC) * scale
    bk_sa = bqkv_h[gh, HD:2 * HD].reshape(DC)
    bv_sa = bqkv_h[gh, 2 * HD:3 * HD].reshape(DC)
    wva_sa, bva_sa = aug_v(wv_sa, bv_sa)
    wvx_c, bvx_c = aug_v(Wv[:, hsl], bv[hsl])
    rank0 = np.float32(1.0 if r == 0 else 0.0)
    return {
        "xT": np.ascontiguousarray(tgt[b].T, f32),
        "memT": np.ascontiguousarray(memory[b].T).astype(bf),
        "wq": np.ascontiguousarray(wq_sa, f32),
        "wk": np.ascontiguousarray(wk_sa, f32),
        "wv": wva_sa,
        "bva": bva_sa,
        "wo": np.ascontiguousarray(Wo_sa[hsl, :]).astype(bf),
        "wqx": np.ascontiguousarray(Wq[:, hsl] * scale, f32),
        "wkx": np.ascontiguousarray(Wk[:, hsl]).astype(bf),
        "wvx": wvx_c.astype(bf),
        "bvxa": bvx_c,
        "wox": np.ascontiguousarray(Wo_cx[hsl, :]).astype(bf),
        "w1": np.ascontiguousarray(W1[:, fsl], f32),
        "w2": np.ascontiguousarray(W2[fsl, :]).astype(bf),
        "biases": pack_biases([
            bq_sa, bk_sa, bq[hsl] * scale, bk[hsl],
            bo_sa * rank0, bo_cx * rank0, b1[fsl], b2 * rank0,
            np.concatenate([g_mha, g_crx, g_ffn]),
            np.concatenate([bn_mha, bn_crx, bn_ffn]),
        ]),
        "ones": np.full((128, 128), 1.0 / D, f32),
    }


def kernel(**inputs):
    tgt = np.asarray(inputs["tgt"], np.float32)
    memory = np.asarray(inputs["memory"], np.float32)
    mask = np.asarray(inputs["tgt_mask"])
    S, M = tgt.shape[1], memory.shape[1]

    if mask.any():
        expect = np.triu(np.ones((S, S), bool), 1)
        if not np.array_equal(mask, expect):
            raise NotImplementedError("only causal or empty tgt_mask supported")
        causal = True
    else:
        causal = False

    nc = _get_nc(S, M, causal)
    args = {k: np.asarray(v, np.float32) for k, v in inputs.items()
            if k not in ("tgt", "memory", "tgt_mask")}
    in_maps = [_prep_inputs(c, S, M, tgt, memory, **args) for c in range(NCORES)]

    trace = bool(int(os.environ.get("BASS_KERNEL_TRACE", "0")))
    res = run_bass_kernel_spmd(nc, in_maps, list(range(NCORES)), trace=trace)
    if trace:
        kernel.last_exec_time_ns = res.exec_time_ns
    out = np.stack([
        np.ascontiguousarray(res.results[0]["outT"].T),
        np.ascontiguousarray(res.results[TP]["outT"].T),
    ])
    return out.astype(np.float32)


# revision 51
# speedup vs baseline: 1.3562x; 1.0784x over previous
"""Trainium2 Bass kernel for nn_DecoderBlock (self-attn + cross-attn + FFN, post-LN).

Sharding: data-parallel over batch (2 groups of 4 cores), tensor-parallel over
heads / FFN hidden dim within each group. Three AllReduces per group, chunked
into token-halves and software-pipelined so each AllReduce overlaps the other
half's compute (including the next stage's work for the already-reduced half).

Precision: fp32 residual stream + LayerNorm; bf16 weights/attention/FFN-hidden
and bf16 AllReduce payloads (validated ~2e-3 rel err vs fp64 reference).

All on-device activations are feature-major ([features on partitions, tokens
on free axis]). The host pre-transposes tgt/memory and post-transposes out.
"""

import os
import sys

sys.path.insert(0, "/opt/trn_rl_repo")

from contextlib import ExitStack

import numpy as np
import ml_dtypes

import concourse.bacc as bacc
import concourse.tile as tile
from concourse import mybir
from concourse.bass_utils import run_bass_kernel_spmd

F32R = mybir.dt.float32r
F32 = mybir.dt.float32
BF16 = mybir.dt.bfloat16
AF = mybir.ActivationFunctionType
ALU = mybir.AluOpType

B = 2
D = 1024
H = 16
HD = 64
FF = 4 * D
NCORES = 8
TP = 4
HL = H // TP          # 4 local heads
DC = HL * HD          # 256 local q/k/v features
DCA = HL * (HD + 1)   # 260: V augmented with a ones column per head
FFC = FF // TP        # 1024 local ffn features
GROUPS = [[0, 1, 2, 3], [4, 5, 6, 7]]
DT = D // 128         # 8 feature partition-tiles
NF = FFC // 128       # 8 ffn partition-tiles

_nc_cache = {}


def _build(S, M, causal):
    nc = bacc.Bacc(None, target_bir_lowering=False, num_devices=NCORES)

    SH = S // 2           # tokens per pipeline half
    CW = 512              # chunk width
    NCH = SH // CW        # chunks per half
    KPC = CW // 128       # key tiles per chunk width

    # ---- DRAM parameters ----
    dp = nc.declare_dram_parameter
    xT = dp("xT", [D, S], F32R, isOutput=False)
    memT = dp("memT", [D, M], BF16, isOutput=False)
    wq = dp("wq", [D, DC], F32R, isOutput=False)
    wk = dp("wk", [D, DC], F32R, isOutput=False)
    wv = dp("wv", [D, DCA], F32R, isOutput=False)
    bva = dp("bva", [1, DCA], F32, isOutput=False)
    wo = dp("wo", [DC, D], BF16, isOutput=False)
    wqx = dp("wqx", [D, DC], F32R, isOutput=False)
    wkx = dp("wkx", [D, DC], BF16, isOutput=False)
    wvx = dp("wvx", [D, DCA], BF16, isOutput=False)
    bvxa = dp("bvxa", [1, DCA], F32, isOutput=False)
    wox = dp("wox", [DC, D], BF16, isOutput=False)
    w1 = dp("w1", [D, FFC], F32R, isOutput=False)
    w2 = dp("w2", [FFC, D], BF16, isOutput=False)
    # packed [128,1] bias columns: bq(2) bk(2) bqx(2) bkx(2) bo(8) box(8)
    # b1(8) b2(8) lng(24) lnb(24)
    biases = dp("biases", [128, 88], F32, isOutput=False)
    ones = dp("ones", [128, 128], F32R, isOutput=False)
    outT = dp("outT", [D, S], F32R, isOutput=True)

    with tile.TileContext(nc) as tc, ExitStack() as st:
        ep = st.enter_context
        constp = ep(tc.tile_pool(name="const", bufs=1))
        aggp = ep(tc.tile_pool(name="agg", bufs=8))
        wf32p = ep(tc.tile_pool(name="wf32", bufs=8))
        wobigp = ep(tc.tile_pool(name="wobig", bufs=2))
        wsmp = ep(tc.tile_pool(name="wsm", bufs=16))
        w2p = ep(tc.tile_pool(name="w2p", bufs=8))
        qkp = ep(tc.tile_pool(name="qk", bufs=4))
        vp = ep(tc.tile_pool(name="vp", bufs=17))
        memp = ep(tc.tile_pool(name="memp", bufs=8))
        hp = ep(tc.tile_pool(name="hp", bufs=8))
        app = ep(tc.tile_pool(name="ap", bufs=3))
        ppool = ep(tc.tile_pool(name="pp", bufs=4))
        zpp = ep(tc.tile_pool(name="zpp", bufs=2))
        zrp = ep(tc.tile_pool(name="zrp", bufs=2))
        lnp = ep(tc.tile_pool(name="lnp", bufs=1))
        dnp = ep(tc.tile_pool(name="dnp", bufs=2))
        dramp = ep(tc.tile_pool(name="dram", bufs=1, space="DRAM"))
        ps_proj = ep(tc.tile_pool(name="psP", bufs=2, space="PSUM"))
        ps_sc = ep(tc.tile_pool(name="psS", bufs=2, space="PSUM"))
        ps_pv = ep(tc.tile_pool(name="psV", bufs=2, space="PSUM"))
        ps_out = ep(tc.tile_pool(name="psO", bufs=2, space="PSUM"))

        # After the first collective trigger, gpsimd carries ONLY triggers
        # (they execute synchronously and head-of-line block anything queued
        # behind them). Prologue DMAs may still use it.
        DMA_ENG = [nc.sync, nc.scalar]
        DMA_ENG3 = [nc.sync, nc.scalar, nc.gpsimd]

        def dma_spread(out_ap, in_ap, nsplit=4, engines=None):
            engines = engines or DMA_ENG
            p = out_ap.shape[0]
            step = p // nsplit
            for i in range(nsplit):
                sl = slice(i * step, (i + 1) * step if i < nsplit - 1 else p)
                engines[i % len(engines)].dma_start(out=out_ap[sl],
                                                    in_=in_ap[sl])

        # ---- constants ----
        ones_t = constp.tile([128, 128], F32R, name="ones_t")
        nc.scalar.dma_start(out=ones_t[:], in_=ones[:, :])
        eps_t = constp.tile([128, 1], F32, name="eps_t")
        nc.vector.memset(eps_t[:], 1e-5)

        ball = constp.tile([128, 88], F32, name="ball")
        nc.sync.dma_start(out=ball[:], in_=biases[:, :])

        def bias_tiles(col0, n):
            return [ball[:, col0 + i:col0 + i + 1] for i in range(n)]

        bq_t = bias_tiles(0, 2)
        bk_t = bias_tiles(2, 2)
        bqx_t = bias_tiles(4, 2)
        bkx_t = bias_tiles(6, 2)
        bo_t = bias_tiles(8, DT)
        box_t = bias_tiles(16, DT)
        b1_t = bias_tiles(24, NF)
        b2_t = bias_tiles(32, DT)
        lng_t = bias_tiles(40, 3 * DT)
        lnb_t = bias_tiles(64, 3 * DT)
        bva_t = constp.tile([128, DCA], F32, name="bva_t")
        nc.scalar.dma_start(out=bva_t[:], in_=bva[:, :].to_broadcast([128, DCA]))
        bvxa_t = constp.tile([128, DCA], F32, name="bvxa_t")
        nc.scalar.dma_start(out=bvxa_t[:],
                            in_=bvxa[:, :].to_broadcast([128, DCA]))

        # ones row at partition 64 for the K=1 denominator-broadcast matmul
        ones65 = constp.tile([65, 64], F32R, name="ones65")
        nc.vector.memset(ones65[:].bitcast(F32), 1.0)

        # 4 static causal mask tiles (prologue gpsimd use is safe: no
        # collective has been triggered yet). mask_j[k, q] = (q - 128j >= k).
        mask_t = []
        if causal:
            mones = constp.tile([128, CW], BF16, name="mones")
            nc.vector.memset(mones[:], 1.0)
            for j in range(KPC):
                mt = constp.tile([128, CW], BF16, name=f"mask{j}")
                nc.gpsimd.affine_select(
                    out=mt[:], in_=mones[:], pattern=[[1, CW]],
                    compare_op=ALU.is_ge, fill=0.0,
                    base=-128 * j, channel_multiplier=-1)
                mask_t.append(mt)

        # ---- DRAM bounce buffers: unique tag per tensor (no aliasing) ----
        ccbuf = [[(dramp.tile([D, SH], BF16, name=f"zin{s_}{h_}",
                              tag=f"zin{s_}{h_}"),
                   dramp.tile([D, SH], BF16, name=f"zout{s_}{h_}",
                              tag=f"zout{s_}{h_}"))
                  for h_ in range(2)] for s_ in range(3)]

        # ================= helpers =================
        def load_wf32(src, cols, tag="w32", engines=None):
            """8 [128, cols] f32r tiles from the shared f32 weight ring."""
            engines = engines or DMA_ENG
            ts = []
            for d in range(DT):
                t = wf32p.tile([128, cols], F32R, name=tag, tag="w32", bufs=8)
                engines[d % len(engines)].dma_start(
                    out=t[:], in_=src[d * 128:(d + 1) * 128, :])
                ts.append(t)
            return ts

        def load_wbf(pool, src, rows, cols, tag, bufs, engines=None):
            """rows//128 [128, cols] bf16 tiles from a bf16 weight ring."""
            engines = engines or DMA_ENG
            ts = []
            for d in range(rows // 128):
                t = pool.tile([128, cols], BF16, name=tag, tag=tag, bufs=bufs)
                engines[d % len(engines)].dma_start(
                    out=t[:], in_=src[d * 128:(d + 1) * 128, :])
                ts.append(t)
            return ts

        def project_qk(qt_pair, w_tiles, b_tiles, src_tiles, tag):
            """Full-S q/k projection into 2 packed [128, S] bf16 tiles."""
            for t in range(2):
                for sc in range(S // CW):
                    sl = slice(sc * CW, (sc + 1) * CW)
                    ps = ps_proj.tile([128, CW], F32, name=f"{tag}ps",
                                      tag="psP", bufs=2)
                    for d in range(DT):
                        nc.tensor.matmul(
                            out=ps[:],
                            lhsT=w_tiles[d][:, t * 128:(t + 1) * 128],
                            rhs=src_tiles[d][:, sl],
                            start=(d == 0), stop=(d == DT - 1),
                        )
                    nc.vector.tensor_scalar_add(qt_pair[t][:, sl], ps[:],
                                                b_tiles[t][:])

        def v_project_sa(w_tiles, src_tiles):
            """Self-attn V: token-major [128, DCA] bf16 tiles, one per 128 toks."""
            vs = []
            for s_t in range(S // 128):
                ps = ps_pv.tile([128, DCA], F32, name="vps", tag="psV", bufs=2)
                for d in range(DT):
                    nc.tensor.matmul(
                        out=ps[:],
                        lhsT=src_tiles[d][:, s_t * 128:(s_t + 1) * 128],
                        rhs=w_tiles[d][:],
                        start=(d == 0), stop=(d == DT - 1))
                vt = vp.tile([128, DCA], BF16, name="vs", tag="v", bufs=18)
                nc.vector.tensor_add(out=vt[:], in0=ps[:], in1=bva_t[:])
                vs.append(vt)
            return vs

        def attention_half(q_pair, k_pair, v_tiles, n_keys, use_mask, th, tag):
            """Attention for token-half th -> 2 packed [128, SH] bf16 tiles."""
            a_packed = [app.tile([128, SH], BF16, name=f"{tag}{t}", tag="attn",
                                 bufs=3) for t in range(2)]
            kt_total = n_keys // 128
            for qc in range(th * NCH, (th + 1) * NCH):
                for h in range(HL):
                    par, ti = h % 2, h // 2
                    kts = range(min(kt_total, KPC * (qc + 1)) if use_mask
                                else kt_total)
                    n_kt = len(kts)
                    pv_ps = ps_pv.tile([65, CW], F32, name=f"{tag}pv",
                                       tag="psV", bufs=2)
                    for i, kt in enumerate(kts):
                        s_ps = ps_sc.tile([128, CW], F32, name=f"{tag}s",
                                          tag="psS", bufs=2)
                        nc.tensor.matmul(
                            out=s_ps[:],
                            lhsT=k_pair[ti][par * 64:(par + 1) * 64,
                                            kt * 128:(kt + 1) * 128],
                            rhs=q_pair[ti][par * 64:(par + 1) * 64,
                                           qc * CW:(qc + 1) * CW],
                            start=True, stop=True,
                        )
                        p_t = ppool.tile([128, CW], BF16, name=f"{tag}p",
                                         tag="p", bufs=3)
                        nc.scalar.activation(out=p_t[:], in_=s_ps[:], func=AF.Exp)
                        if use_mask and kt >= KPC * qc:
                            p_m = ppool.tile([128, CW], BF16, name=f"{tag}pm",
                                             tag="p", bufs=3)
                            nc.vector.tensor_mul(out=p_m[:], in0=p_t[:],
                                                 in1=mask_t[kt - KPC * qc][:])
                            p_use = p_m
                        else:
                            p_use = p_t
                        nc.tensor.matmul(
                            out=pv_ps[:],
                            lhsT=v_tiles[kt][:, h * 65:(h + 1) * 65],
                            rhs=p_use[:],
                            start=(i == 0), stop=(i == n_kt - 1),
                        )
                    # softmax denominator: reciprocal on row 64, then
                    # broadcast down 64 partitions via a K=1 matmul.
                    dsc = dnp.tile([65, CW], F32R, name=f"{tag}dsc", tag="dsc",
                                   bufs=2)
                    with nc.allow_low_precision(
                            reason="softmax denom reciprocal feeds an f32r "
                                   "broadcast matmul; f32r rounding ~1e-5"):
                        nc.vector.reciprocal(out=dsc[64:65, :],
                                             in_=pv_ps[64:65, :])
                    db_ps = ps_pv.tile([64, CW], F32, name=f"{tag}dbp",
                                       tag="psV", bufs=2)
                    nc.tensor.matmul(out=db_ps[:], lhsT=ones65[64:65, :],
                                     rhs=dsc[64:65, :], start=True, stop=True)
                    db = dnp.tile([64, CW], F32, name=f"{tag}db", tag="db",
                                  bufs=2)
                    nc.vector.tensor_scalar_add(db[:], db_ps[:], 0.0)
                    lc = qc - th * NCH
                    sl = slice(lc * CW, (lc + 1) * CW)
                    if par == 0:
                        nc.vector.tensor_mul(out=a_packed[ti][0:64, sl],
                                             in0=pv_ps[0:64, :], in1=db[:])
                    else:
                        sh = dnp.tile([64, CW], BF16, name=f"{tag}sh",
                                      tag="sh", bufs=1)
                        nc.vector.tensor_mul(out=sh[:], in0=pv_ps[0:64, :],
                                             in1=db[:])
                        nc.sync.dma_start(out=a_packed[ti][64:128, sl], in_=sh[:])
            return a_packed

        def out_project_spill(wo_t, a_packed, bo_tiles, stage, th, tag):
            """y = wo.T @ attn for half th; bf16 spill + AllReduce trigger."""
            zin, zout = ccbuf[stage][th]
            for d in range(DT):
                zp = zpp.tile([128, SH], BF16, name=f"{tag}zp", tag="zp",
                              bufs=2)
                for c in range(NCH):
                    ps = ps_out.tile([128, CW], F32, name=f"{tag}ps",
                                     tag="psO", bufs=2)
                    for ct in range(2):
                        nc.tensor.matmul(
                            out=ps[:],
                            lhsT=wo_t[ct][:, d * 128:(d + 1) * 128],
                            rhs=a_packed[ct][:, c * CW:(c + 1) * CW],
                            start=(ct == 0), stop=(ct == 1),
                        )
                    nc.scalar.activation(
                        out=zp[:, c * CW:(c + 1) * CW], in_=ps[:],
                        func=AF.Identity, bias=bo_tiles[d][:], scale=1.0)
                dma_spread(zin[d * 128:(d + 1) * 128, :], zp[:])
            nc.gpsimd.collective_compute(
                "AllReduce", ALU.add, replica_groups=GROUPS,
                ins=[zin.opt()], outs=[zout.opt()])

        def reload_add(agg, stage, th, tag):
            """agg[:, half] += allreduced partial (bf16 -> fp32 accumulate).

            The zr DMAs wait on the AllReduce; on sync/scalar the scheduler
            hoists them ahead of non-AR-dependent work and head-of-line
            blocks it. gpsimd's queue (triggers only) is the natural block
            point for h0; sync tolerates the h1 wait (nothing the parallel
            compute needs sits behind it)."""
            eng = nc.gpsimd if th == 0 else nc.sync
            hs = slice(th * SH, (th + 1) * SH)
            zout = ccbuf[stage][th][1]
            for d in range(DT):
                zr = zrp.tile([128, SH], BF16, name=f"{tag}zr", tag="zr",
                              bufs=2)
                for i in range(2):
                    eng.dma_start(
                        out=zr[i * 64:(i + 1) * 64, :],
                        in_=zout[d * 128 + i * 64:d * 128 + (i + 1) * 64, :])
                zf = zrp.tile([128, SH], F32, name=f"{tag}zf", tag="zf",
                              bufs=2)
                nc.scalar.copy(out=zf[:], in_=zr[:])
                nc.vector.tensor_tensor(out=agg[d][:, hs], in0=zf[:],
                                        in1=agg[d][:, hs].bitcast(F32),
                                        op=ALU.add)

        def ln_half(agg, ln_idx, th, tag):
            """LayerNorm (fp32) over features for token-half th, in place."""
            hs = slice(th * SH, (th + 1) * SH)
            mu = lnp.tile([128, SH], F32, name=f"{tag}mu", tag="mu", bufs=1)
            std = lnp.tile([128, SH], F32, name=f"{tag}st", tag="st", bufs=1)
            rstd = lnp.tile([128, SH], F32, name=f"{tag}rs", tag="rs", bufs=1)
            for c in range(NCH):
                gc = th * NCH + c
                sl = slice(gc * CW, (gc + 1) * CW)
                lsl = slice(c * CW, (c + 1) * CW)
                mps = ps_proj.tile([128, CW], F32, name=f"{tag}m", tag="psP",
                                   bufs=2)
                for d in range(DT):
                    nc.tensor.matmul(out=mps[:], lhsT=ones_t[:],
                                     rhs=agg[d][:, sl],
                                     start=(d == 0), stop=(d == DT - 1))
                nc.scalar.copy(out=mu[:, lsl], in_=mps[:])
                qps = ps_proj.tile([128, CW], F32, name=f"{tag}q", tag="psP",
                                   bufs=2)
                for d in range(DT):
                    sq = lnp.tile([128, CW], F32R, name=f"{tag}sq", tag="sq",
                                  bufs=2)
                    nc.vector.tensor_mul(out=sq[:],
                                         in0=agg[d][:, sl].bitcast(F32),
                                         in1=agg[d][:, sl].bitcast(F32))
                    nc.tensor.matmul(out=qps[:], lhsT=ones_t[:], rhs=sq[:],
                                     start=(d == 0), stop=(d == DT - 1))
                musq = lnp.tile([128, CW], F32, name=f"{tag}m2", tag="m2",
                                bufs=1)
                nc.vector.tensor_mul(out=musq[:], in0=mu[:, lsl],
                                     in1=mu[:, lsl])
                var = lnp.tile([128, CW], F32, name=f"{tag}v", tag="var",
                               bufs=1)
                nc.vector.tensor_tensor(out=var[:], in0=qps[:], in1=musq[:],
                                        op=ALU.subtract)
                nc.scalar.activation(out=std[:, lsl], in_=var[:],
                                     func=AF.Sqrt, bias=eps_t[:], scale=1.0)
            nc.vector.reciprocal_approx_fast(out=rstd[:], in_=std[:])
            for d in range(DT):
                xm = lnp.tile([128, SH], F32, name=f"{tag}x", tag="xm", bufs=1)
                nc.vector.tensor_tensor(out=xm[:],
                                        in0=agg[d][:, hs].bitcast(F32),
                                        in1=mu[:], op=ALU.subtract)
                nc.vector.tensor_mul(out=xm[:], in0=xm[:], in1=rstd[:])
                nc.vector.tensor_scalar(
                    out=agg[d][:, hs], in0=xm[:],
                    scalar1=lng_t[ln_idx * DT + d][:],
                    scalar2=lnb_t[ln_idx * DT + d][:],
                    op0=ALU.mult, op1=ALU.add)

        # ================= pipeline =================
        # ---- P0/P1: load x + SA weights, project q/k/v ----
        # interleave x and q/k weight loads so the first projection chain
        # (which needs ALL x tiles + all wq tiles) is fed as fast as possible
        agg = []
        wq_t = []
        for d in range(DT):
            t = aggp.tile([128, S], F32R, name="agg", tag="agg", bufs=8)
            dma_spread(t[:], xT[d * 128:(d + 1) * 128, :], nsplit=2)
            agg.append(t)
            w = wf32p.tile([128, DC], F32R, name="wqt", tag="w32", bufs=8)
            nc.gpsimd.dma_start(out=w[:], in_=wq[d * 128:(d + 1) * 128, :])
            wq_t.append(w)
        wk_t = load_wf32(wk, DC, "wkt", DMA_ENG3)

        q_s = [qkp.tile([128, S], BF16, name="qs", tag="qk", bufs=4)
               for _ in range(2)]
        k_s = [qkp.tile([128, S], BF16, name="ks", tag="qk", bufs=4)
               for _ in range(2)]
        project_qk(q_s, wq_t, bq_t, agg, "qs")
        project_qk(k_s, wk_t, bk_t, agg, "ks")
        wv_t = load_wf32(wv, DCA, "wvt", DMA_ENG3)
        wo_t = load_wbf(wobigp, wo, DC, D, "wob", 2, DMA_ENG3)
        v_s = v_project_sa(wv_t, agg)

        # ---- P2/P3: self attention per half, spill + AR ----
        for th in range(2):
            a_h = attention_half(q_s, k_s, v_s, S, causal, th, "sa")
            out_project_spill(wo_t, a_h, bo_t, 0, th, "o1")

        # ---- P4: cross K/V from one pass over memory ----
        wkx_t = load_wbf(wsmp, wkx, D, DC, "wsm", 16)
        wvx_t = load_wbf(wsmp, wvx, D, DCA, "wsm", 16)
        wqx_t = load_wf32(wqx, DC, "wqxt")
        k_x = [qkp.tile([128, M], BF16, name="kx", tag="qk", bufs=4)
               for _ in range(2)]
        v_x = []
        for sc in range(M // CW):
            mts = []
            for d in range(DT):
                mt = memp.tile([128, CW], BF16, name="memt", tag="mem", bufs=9)
                nc.sync.dma_start(
                    out=mt[:], in_=memT[d * 128:(d + 1) * 128,
                                        sc * CW:(sc + 1) * CW])
                mts.append(mt)
            for t in range(2):
                ps = ps_proj.tile([128, CW], F32, name="kxps", tag="psP",
                                  bufs=2)
                for d in range(DT):
                    nc.tensor.matmul(
                        out=ps[:],
                        lhsT=wkx_t[d][:, t * 128:(t + 1) * 128],
                        rhs=mts[d][:],
                        start=(d == 0), stop=(d == DT - 1))
                nc.scalar.activation(
                    out=k_x[t][:, sc * CW:(sc + 1) * CW], in_=ps[:],
                    func=AF.Identity, bias=bkx_t[t][:], scale=1.0)
            for j in range(KPC):
                ps = ps_pv.tile([128, DCA], F32, name="vxps", tag="psV",
                                bufs=2)
                for d in range(DT):
                    nc.tensor.matmul(
                        out=ps[:],
                        lhsT=mts[d][:, j * 128:(j + 1) * 128],
                        rhs=wvx_t[d][:],
                        start=(d == 0), stop=(d == DT - 1))
                vt = vp.tile([128, DCA], BF16, name="vx", tag="v", bufs=18)
                nc.vector.tensor_add(out=vt[:], in0=ps[:], in1=bvxa_t[:])
                v_x.append(vt)

        # ---- P5..P7: LN1 + cross attention, per half (pipelined vs ARs) ----
        wox_t = load_wbf(wobigp, wox, DC, D, "wob", 2)
        q_x = [qkp.tile([128, S], BF16, name="qx", tag="qk", bufs=4)
               for _ in range(2)]

        def qx_project_half(th):
            for t in range(2):
                for c in range(NCH):
                    gc = th * NCH + c
                    sl = slice(gc * CW, (gc + 1) * CW)
                    ps = ps_proj.tile([128, CW], F32, name="qxps", tag="psP",
                                      bufs=2)
                    for d in range(DT):
                        nc.tensor.matmul(
                            out=ps[:],
                            lhsT=wqx_t[d][:, t * 128:(t + 1) * 128],
                            rhs=agg[d][:, sl],
                            start=(d == 0), stop=(d == DT - 1))
                    nc.vector.tensor_scalar_add(q_x[t][:, sl], ps[:],
                                                bqx_t[t][:])

        for th in range(2):
            reload_add(agg, 0, th, "l1")
            ln_half(agg, 0, th, "l1")
            qx_project_half(th)
            a_h = attention_half(q_x, k_x, v_x, M, False, th, "cx")
            out_project_spill(wox_t, a_h, box_t, 1, th, "o2")

        # ---- P8/P9: LN2 + FFN per half ----
        for th in range(2):
            reload_add(agg, 1, th, "l2")
            ln_half(agg, 1, th, "l2")
            h_fm = []
            # FFN1 in four f-groups so only 8 w1 tiles are live at a time
            for fc in range(4):
                w1_t = []  # [128, 256] f32r tiles: all d rows, fc col-group
                for d in range(DT):
                    t = wf32p.tile([128, 256], F32R, name="w1t", tag="w32",
                                   bufs=8)
                    DMA_ENG[d % 2].dma_start(
                        out=t[:], in_=w1[d * 128:(d + 1) * 128,
                                         fc * 256:(fc + 1) * 256])
                    w1_t.append(t)
                for f in range(fc * 2, (fc + 1) * 2):
                    ht = hp.tile([128, SH], BF16, name=f"hfm{f}", tag="hfm",
                                 bufs=8)
                    for c in range(NCH):
                        gc = th * NCH + c
                        sl = slice(gc * CW, (gc + 1) * CW)
                        ps = ps_out.tile([128, CW], F32, name="f1ps",
                                         tag="psO", bufs=2)
                        for d in range(DT):
                            nc.tensor.matmul(
                                out=ps[:],
                                lhsT=w1_t[d][:, (f % 2) * 128:
                                             (f % 2 + 1) * 128],
                                rhs=agg[d][:, sl],
                                start=(d == 0), stop=(d == DT - 1))
                        nc.scalar.activation(out=ht[:, c * CW:(c + 1) * CW],
                                             in_=ps[:], func=AF.Relu,
                                             bias=b1_t[f][:], scale=1.0)
                    h_fm.append(ht)
            # FFN2 in four d-groups so only 8 w2 tiles are live at a time
            zin, zout = ccbuf[2][th]
            for dc2 in range(4):
                w2_t = []  # [128, 256] bf16 tiles: all f rows, dc2 col-group
                for f in range(NF):
                    t = w2p.tile([128, 256], BF16, name="w2t", tag="w2",
                                 bufs=8)
                    DMA_ENG[f % 2].dma_start(
                        out=t[:], in_=w2[f * 128:(f + 1) * 128,
                                         dc2 * 256:(dc2 + 1) * 256])
                    w2_t.append(t)
                for d in range(dc2 * 2, (dc2 + 1) * 2):
                    zp = zpp.tile([128, SH], BF16, name="f2zp", tag="zp",
                                  bufs=2)
                    for c in range(NCH):
                        ps = ps_out.tile([128, CW], F32, name="f2ps",
                                         tag="psO", bufs=2)
                        for f in range(NF):
                            nc.tensor.matmul(
                                out=ps[:],
                                lhsT=w2_t[f][:, (d % 2) * 128:
                                             (d % 2 + 1) * 128],
                                rhs=h_fm[f][:, c * CW:(c + 1) * CW],
                                start=(f == 0), stop=(f == NF - 1))
                        nc.scalar.activation(
                            out=zp[:, c * CW:(c + 1) * CW], in_=ps[:],
                            func=AF.Identity, bias=b2_t[d][:], scale=1.0)
                    dma_spread(zin[d * 128:(d + 1) * 128, :], zp[:])
            nc.gpsimd.collective_compute(
                "AllReduce", ALU.add, replica_groups=GROUPS,
                ins=[zin.opt()], outs=[zout.opt()])

        # ---- P10/P11: LN3 + output (stores on scalar so the h1 reload's
        # AR-wait on sync can't delay the h0 store) ----
        for th in range(2):
            reload_add(agg, 2, th, "l3")
            ln_half(agg, 2, th, "l3")
            hs = slice(th * SH, (th + 1) * SH)
            for d in range(DT):
                dma_spread(outT[d * 128:(d + 1) * 128, hs], agg[d][:, hs],
                           nsplit=2, engines=[nc.scalar])

    nc.finalize()
    return nc


def _get_nc(S, M, causal):
    key = (S, M, causal)
    if key not in _nc_cache:
        _nc_cache[key] = _build(S, M, causal)
    return _nc_cache[key]


def _prep_inputs(c, S, M, tgt, memory, Wqkv, bqkv, Wo_sa, bo_sa, Wq, bq, Wk, bk,
                 Wv, bv, Wo_cx, bo_cx, W1, b1, W2, b2, g_mha, bn_mha, g_crx,
                 bn_crx, g_ffn, bn_ffn):
    r, b = c % TP, c // TP
    hsl = slice(r * DC, (r + 1) * DC)
    fsl = slice(r * FFC, (r + 1) * FFC)
    f32 = np.float32
    bf = ml_dtypes.bfloat16

    def aug_v(wv_c, bv_c):
        wva = np.zeros((D, DCA), f32)
        bva = np.zeros((1, DCA), f32)
        for h in range(HL):
            wva[:, h * 65:h * 65 + 64] = wv_c[:, h * 64:(h + 1) * 64]
            bva[0, h * 65:h * 65 + 64] = bv_c[h * 64:(h + 1) * 64]
            bva[0, h * 65 + 64] = 1.0
        return wva, bva

    def pack_biases(cols):
        out = np.zeros((128, 88), f32)
        i = 0
        for v in cols:
            v = np.asarray(v, f32).reshape(-1)
            n = len(v) // 128
            for j in range(n):
                out[:, i] = v[j * 128:(j + 1) * 128]
                i += 1
        assert i == 88
        return out

    scale = np.float32(1.0 / np.sqrt(HD))
    # Wqkv columns are per-head interleaved: head g = cols g*192 + [q64|k64|v64]
    wqkv_h = Wqkv.reshape(D, H, 3 * HD)
    bqkv_h = bqkv.reshape(H, 3 * HD)
    gh = slice(r * HL, (r + 1) * HL)  # this rank's global heads
    wq_sa = wqkv_h[:, gh, 0:HD].reshape(D, DC) * scale
    wk_sa = wqkv_h[:, gh, HD:2 * HD].reshape(D, DC)
    wv_sa = wqkv_h[:, gh, 2 * HD:3 * HD].reshape(D, DC)
    bq_sa = bqkv_h[gh, 0:HD].reshape(DC) * scale
    bk_sa = bqkv_h[gh, HD:2 * HD].reshape(DC)
    bv_sa = bqkv_h[gh, 2 * HD:3 * HD].reshape(DC)
    wva_sa, bva_sa = aug_v(wv_sa, bv_sa)
    wvx_c, bvx_c = aug_v(Wv[:, hsl], bv[hsl])
    rank0 = np.float32(1.0 if r == 0 else 0.0)
    return {
        "xT": np.ascontiguousarray(tgt[b].T, f32),
        "memT": np.ascontiguousarray(memory[b].T).astype(bf),
        "wq": np.ascontiguousarray(wq_sa, f32),
        "wk": np.ascontiguousarray(wk_sa, f32),
        "wv": wva_sa,
        "bva": bva_sa,
        "wo": np.ascontiguousarray(Wo_sa[hsl, :]).astype(bf),
        "wqx": np.ascontiguousarray(Wq[:, hsl] * scale, f32),
        "wkx": np.ascontiguousarray(Wk[:, hsl]).astype(bf),
        "wvx": wvx_c.astype(bf),
        "bvxa": bvx_c,
        "wox": np.ascontiguousarray(Wo_cx[hsl, :]).astype(bf),
        "w1": np.ascontiguousarray(W1[:, fsl], f32),
        "w2": np.ascontiguousarray(W2[fsl, :]).astype(bf),
        "biases": pack_biases([
            bq_sa, bk_sa, bq[hsl] * scale, bk[hsl],
            bo_sa * rank0, bo_cx * rank0, b1[fsl], b2 * rank0,
            np.concatenate([g_mha, g_crx, g_ffn]),
            np.concatenate([bn_mha, bn_crx, bn_ffn]),
        ]),
        "ones": np.full((128, 128), 1.0 / D, f32),
    }


def kernel(**inputs):
    tgt = np.asarray(inputs["tgt"], np.float32)
    memory = np.asarray(inputs["memory"], np.float32)
    mask = np.asarray(inputs["tgt_mask"])
    S, M = tgt.shape[1], memory.shape[1]

    if mask.any():
        expect = np.triu(np.ones((S, S), bool), 1)
        if not np.array_equal(mask, expect):
            raise NotImplementedError("only causal or empty tgt_mask supported")
        causal = True
    else:
        causal = False

    nc = _get_nc(S, M, causal)
    args = {k: np.asarray(v, np.float32) for k, v in inputs.items()
            if k not in ("tgt", "memory", "tgt_mask")}
    in_maps = [_prep_inputs(c, S, M, tgt, memory, **args) for c in range(NCORES)]

    trace = bool(int(os.environ.get("BASS_KERNEL_TRACE", "0")))
    res = run_bass_kernel_spmd(nc, in_maps, list(range(NCORES)), trace=trace)
    if trace:
        kernel.last_exec_time_ns = res.exec_time_ns
    out = np.stack([
        np.ascontiguousarray(res.results[0]["outT"].T),
        np.ascontiguousarray(res.results[TP]["outT"].T),
    ])
    return out.astype(np.float32)
